# revision 14
# baseline (speedup 1.0000x reference)
"""PlantCCCEncoder (3-layer GAT over 50k nodes / 800k edges) on 8 trn2 cores.

Strategy: edges sharded by dst node range (each core owns 6250 dst nodes and
all edges pointing into them -> segment softmax/sum are core-local, no
all-reduce). Node features replicated via a per-layer AllGather of the
src-projection table XS. Per-edge xs rows fetched with indirect DMA gathers;
xd and edge-feature terms come from PE matmuls (one-hot expansion) off
SBUF-resident tables, overlapped under the gather.

Self-contained: hardcodes all shapes; builds + runs the Bass program via
run_bass_kernel_spmd on cores 0-7.
"""
import sys
import types

sys.path.insert(0, "/opt/trn_rl_repo")

import numpy as np

import concourse.bass as bass
import concourse.tile as tile
from concourse import mybir
from concourse.bass_utils import run_bass_kernel_spmd

# ---------------- model dims (from the reference) ----------------
N = 50000
E = 800000
IN = 256
HID = 128
H = 4
C = 32
L = 3
REL = 100
RD = 16
EH = 32
LN_EPS = 1e-5

NCORES = 8
P = 128
NL = N // NCORES            # 6250 local dst nodes per core
NB = (NL + P - 1) // P      # 49 dst blocks
NLP = NB * P                # 6272 padded local nodes
NG = NCORES * NLP           # 50176 rows in the all-gathered XS table

F32 = mybir.dt.float32
I32 = mybir.dt.int32

PAD_COL = 200.0             # one-hot column id for padding edges (never matches)


# ---------------- NTFF profile hook (for trace timing) ----------------
def _install_ntff_hook():
    try:
        import antenv  # noqa: F401
        if "antenv.axon_hooks" in sys.modules:
            return
        mod = types.ModuleType("antenv.axon_hooks")
        _hook = [None]
        mod.set_axon_ntff_profile_hook = lambda h: _hook.__setitem__(0, h)
        mod.get_axon_ntff_profile_hook = lambda: _hook[0]
        sys.modules["antenv.axon_hooks"] = mod
        setattr(sys.modules["antenv"], "axon_hooks", mod)
        from trn_agent_boot.trn_boot import _ntff_profile_via_ctypes
        mod.set_axon_ntff_profile_hook(
            _ntff_profile_via_ctypes("/opt/axon/libaxon_pjrt.so"))
    except Exception:
        pass


# ---------------- walrus 1-wait-per-instruction workaround ----------------
def _split_waits(nc):
    def _get_si(ins):
        si = ins.sync_info
        return si() if callable(si) else si

    for f in nc.m.functions:
        for bb in f.blocks:
            insts = list(bb.instructions)
            out = []
            changed = False
            for ins in insts:
                if not isinstance(ins, mybir.InstEventSemaphore):
                    si = _get_si(ins)
                    if si is not None and len(si.on_wait) > 1:
                        waits = list(si.on_wait)
                        for k, w in enumerate(waits[1:]):
                            ev = mybir.InstEventSemaphore(
                                name=f"{ins.name}-ws{k}", engine=ins.engine,
                                ins=[], outs=[],
                                sync_info=mybir.SyncInfo(on_wait=[w], on_update=[]))
                            ev.bass_nofuse = True
                            out.append(ev)
                        ins.sync_info = mybir.SyncInfo(
                            on_wait=waits[:1], on_update=list(si.on_update))
                        changed = True
                out.append(ins)
            if changed:
                bb.instructions = out


# ---------------- host-side preprocessing ----------------
def _prep(inputs):
    x = np.asarray(inputs["x"], np.float32)
    edge_index = np.asarray(inputs["edge_index"]).astype(np.int64)
    edge_attr = np.asarray(inputs["edge_attr"], np.float32)

    g = {k: np.asarray(v, np.float32) for k, v in inputs.items()
         if k not in ("x", "edge_index", "edge_attr")}

    src = edge_index[0]
    dst = edge_index[1]

    # ---- parameter folding (host, parameters only) ----
    fus_W1 = g["fus_W1"]                     # [48, 32]
    A, B, Cc = fus_W1[:RD], fus_W1[RD:2 * RD], fus_W1[2 * RD:]
    dW2A = g["dist_W2"] @ A                  # [32, 32]
    cW2B = g["coexp_W2"] @ B                 # [32, 32]
    relp = g["rel_emb"] @ Cc                 # [100, 32]
    b_comb = (g["fus_b1"] + g["dist_b2"] @ A + g["coexp_b2"] @ B)  # [32]
    Wfold = np.stack([g["fus_W2"] @ g["Wedge"][i] for i in range(L)])      # [3,32,128]
    bias_ef = np.stack([g["fus_b2"] @ g["Wedge"][i] for i in range(L)])    # [3,128]

    # ---- edge sharding: sort by dst, shard by dst range, block by 128 ----
    order = np.argsort(dst, kind="stable")
    src_s, dst_s = src[order], dst[order]
    ea_s = edge_attr[order]

    core_of = dst_s // NL
    dl = dst_s % NL
    blk = dl // P
    col = dl % P

    # per (core, block) edge lists
    counts = np.zeros((NCORES, NB), np.int64)
    for k in range(NCORES):
        m = core_of == k
        bb = blk[m]
        cnt = np.bincount(bb, minlength=NB)
        counts[k] = cnt
    T_b = np.maximum(1, ((counts.max(axis=0) + P - 1) // P)).astype(np.int64)  # [NB]
    TT = int(T_b.sum())
    EP = TT * P
    tile_base = np.concatenate([[0], np.cumsum(T_b)])[:-1]  # first tile idx per block

    # slot assignment per core
    rng_rows = np.arange(N, dtype=np.int64)
    pad_row = (rng_rows // NL) * NLP + (rng_rows % NL)      # global node -> XS row

    per_core = []
    for k in range(NCORES):
        m = core_of == k
        e_src = src_s[m]
        e_col = col[m]
        e_blk = blk[m]
        e_d = ea_s[m, 0]
        e_c = ea_s[m, 1]
        e_rel = np.clip(ea_s[m, 2].astype(np.int64), 0, REL - 1)

        srcg = np.zeros(EP, np.int32)
        dstf = np.full(EP, PAD_COL, np.float32)
        logd = np.zeros(EP, np.float32)
        dT = np.zeros(EP, np.float32)
        cT = np.zeros(EP, np.float32)
        relT = np.zeros((32, EP), np.float32)

        # stable order within block
        ob = np.argsort(e_blk, kind="stable")
        e_src, e_col, e_blk = e_src[ob], e_col[ob], e_blk[ob]
        e_d, e_c, e_rel = e_d[ob], e_c[ob], e_rel[ob]
        # slot = (tile_base[b]*128 + position within block's slots)
        start = 0
        for b in range(NB):
            nb_ = counts[k, b]
            sl0 = tile_base[b] * P
            sl = np.arange(nb_) + sl0
            seg = slice(start, start + nb_)
            srcg[sl] = pad_row[e_src[seg]]
            dstf[sl] = e_col[seg]
            logd[sl] = np.log(e_d[seg])
            dT[sl] = e_d[seg]
            cT[sl] = e_c[seg]
            relT[:, sl] = relp[e_rel[seg]].T
            start += nb_

        # [128, TT] layouts: slot = ti*128 + p  ->  [p, ti]
        srcg2 = srcg.reshape(TT, P).T.copy()
        dstf2 = dstf.reshape(TT, P).T.copy()
        logd2 = logd.reshape(TT, P).T.copy()

        xk = np.zeros((NLP, IN), np.float32)
        xk[:NL] = x[k * NL:(k + 1) * NL]

        per_core.append(dict(
            x_in=xk, srcg=srcg2, dstf=dstf2, logd=logd2,
            dT=dT[None, :], cT=cT[None, :], relT=relT,
        ))

    def rep(row):
        return np.tile(np.asarray(row, np.float32)[None, :], (P, 1))

    consts = dict(
        identity=np.eye(P, dtype=np.float32),
        iota_row=np.tile(np.arange(P, dtype=np.float32), (P, 1)),
        ones32=np.ones((32, 1), np.float32),
        ones1x32=np.ones((1, 32), np.float32),
        in_g=rep(g["in_g"]), in_b=rep(g["in_b"]),
        proj_W=g["proj_W"], proj_b=rep(g["proj_b"]),
        dW1=g["dist_W1"], cW1=g["coexp_W1"],
        b1d=g["dist_b1"][:, None], b1c=g["coexp_b1"][:, None],
        dW2A=dW2A, cW2B=cW2B, b_comb=b_comb[:, None],
        fusg=g["fus_ln_g"][:, None], fusb=g["fus_ln_b"][:, None],
        Wsrc=g["Wsrc"], Wdst=g["Wdst"], Wfold=Wfold,
        bias_ef=np.stack([rep(bias_ef[i]) for i in range(L)]),
        att=np.stack([rep(g["att"][i].reshape(H * C)) for i in range(L)]),
        sscale=np.stack([rep(g["sscale"][i]) for i in range(L)]),
        ng=np.stack([rep(g["ng"][i]) for i in range(L)]),
        nb=np.stack([rep(g["nb"][i]) for i in range(L)]),
    )
    meta = dict(T_b=T_b.tolist(), tile_base=tile_base.tolist(), TT=TT, EP=EP)
    return per_core, consts, meta


# ---------------- device program ----------------
def _build(consts, meta):
    T_b, tile_base, TT, EP = (meta["T_b"], meta["tile_base"],
                              meta["TT"], meta["EP"])
    nc = bass.Bass(num_devices=NCORES)

    # per-core inputs
    x_in = nc.declare_dram_parameter("x_in", [NLP, IN], F32, isOutput=False)
    srcg = nc.declare_dram_parameter("srcg", [P, TT], I32, isOutput=False)
    dstf = nc.declare_dram_parameter("dstf", [P, TT], F32, isOutput=False)
    logd = nc.declare_dram_parameter("logd", [P, TT], F32, isOutput=False)
    dT = nc.declare_dram_parameter("dT", [1, EP], F32, isOutput=False)
    cT = nc.declare_dram_parameter("cT", [1, EP], F32, isOutput=False)
    relT = nc.declare_dram_parameter("relT", [32, EP], F32, isOutput=False)
    out = nc.declare_dram_parameter("out", [NLP, HID], F32, isOutput=True)
    import os as _os
    KDEBUG = bool(int(_os.environ.get("KDEBUG", "0")))
    if KDEBUG:
        dbg_h = nc.declare_dram_parameter("dbg_h", [32, EP], F32, isOutput=True)
        dbg_x0 = nc.declare_dram_parameter("dbg_x0", [P, NB, HID], F32, isOutput=True)
        dbg_xs = nc.declare_dram_parameter("dbg_xs", [NG, HID], F32, isOutput=True)
        dbg_xd = nc.declare_dram_parameter("dbg_xd", [P, NB, HID], F32, isOutput=True)
        dbg_x1 = nc.declare_dram_parameter("dbg_x1", [P, NB, HID], F32, isOutput=True)

    ct = {k: nc.inline_tensor(np.ascontiguousarray(v), name=f"c_{k}")
          for k, v in consts.items()}

    with tile.TileContext(nc) as tc:
        import contextlib
        ctx = contextlib.ExitStack()
        with ctx:
            sing = ctx.enter_context(tc.tile_pool(name="sing", bufs=1))
            work = ctx.enter_context(tc.tile_pool(name="work", bufs=3))
            wsm = ctx.enter_context(tc.tile_pool(name="wsm", bufs=3))
            ps_acc = ctx.enter_context(tc.tile_pool(name="psacc", bufs=2, space="PSUM"))
            ps_den = ctx.enter_context(tc.tile_pool(name="psden", bufs=2, space="PSUM"))
            ps_ef = ctx.enter_context(tc.tile_pool(name="psef", bufs=2, space="PSUM"))
            ps_m = ctx.enter_context(tc.tile_pool(name="psm", bufs=2, space="PSUM"))
            ps_tr = ps_m
            dram = ctx.enter_context(tc.tile_pool(name="dram", bufs=1, space="DRAM"))
            encp = ctx.enter_context(tc.tile_pool(name="encp", bufs=2))

            _uid = [0]

            def bc_load(mat_t, parts, width, tag=None):
                """Load a pre-replicated [parts, width] constant."""
                _uid[0] += 1
                t = sing.tile([parts, width], F32, tag=tag or f"bc{_uid[0]}")
                nc.sync.dma_start(out=t[:], in_=mat_t[:, :])
                return t

            # ---- persistent SBUF tiles ----
            identity = sing.tile([P, P], F32)
            nc.sync.dma_start(out=identity[:], in_=ct["identity"][:, :])
            iota_row = sing.tile([P, P], F32)
            nc.sync.dma_start(out=iota_row[:], in_=ct["iota_row"][:, :])
            srcg_sb = sing.tile([P, TT], I32)
            nc.sync.dma_start(out=srcg_sb[:], in_=srcg[:, :])
            dstf_sb = sing.tile([P, TT], F32)
            nc.sync.dma_start(out=dstf_sb[:], in_=dstf[:, :])
            logd_sb = sing.tile([P, TT], F32)
            nc.sync.dma_start(out=logd_sb[:], in_=logd[:, :])

            xT_sb = sing.tile([P, NLP], F32)          # x feat-major (lhsT source)
            xnm_sb = sing.tile([P, NB, HID], F32)     # x node-major  [:, b, :]
            XD_sb = sing.tile([P, NB, HID], F32)      # XD' node-major

            in_g_bc = bc_load(ct["in_g"], P, IN)
            in_b_bc = bc_load(ct["in_b"], P, IN)
            proj_b_bc = bc_load(ct["proj_b"], P, HID)
            projW_lo = sing.tile([P, HID], F32)
            nc.sync.dma_start(out=projW_lo[:], in_=ct["proj_W"][0:P, :])
            projW_hi = sing.tile([P, HID], F32)
            nc.sync.dma_start(out=projW_hi[:], in_=ct["proj_W"][P:IN, :])

            Wsrc_sb, Wdst_sb, Wfold_sb = [], [], []
            att_bc, ss_bc, ng_bc, nb_bc, bef_bc = [], [], [], [], []
            for i in range(L):
                t = sing.tile([HID, HID], F32, tag=f"wsrc{i}")
                nc.sync.dma_start(out=t[:], in_=ct["Wsrc"][i, :, :])
                Wsrc_sb.append(t)
                t = sing.tile([HID, HID], F32, tag=f"wdst{i}")
                nc.sync.dma_start(out=t[:], in_=ct["Wdst"][i, :, :])
                Wdst_sb.append(t)
                t = sing.tile([32, HID], F32, tag=f"wfold{i}")
                nc.sync.dma_start(out=t[:], in_=ct["Wfold"][i, :, :])
                Wfold_sb.append(t)
                att_bc.append(bc_load(ct["att"][i], P, H * C))
                ss_bc.append(bc_load(ct["sscale"][i], P, H))
                ng_bc.append(bc_load(ct["ng"][i], P, HID))
                nb_bc.append(bc_load(ct["nb"][i], P, HID))
                bef_bc.append(bc_load(ct["bias_ef"][i], P, HID))

            # encoder consts (feat-major; per-partition columns)
            dW1_sb = sing.tile([1, 32], F32)
            nc.sync.dma_start(out=dW1_sb[:], in_=ct["dW1"][:, :])
            cW1_sb = sing.tile([1, 32], F32)
            nc.sync.dma_start(out=cW1_sb[:], in_=ct["cW1"][:, :])
            b1d_sb = sing.tile([32, 1], F32)
            nc.sync.dma_start(out=b1d_sb[:], in_=ct["b1d"][:, :])
            b1c_sb = sing.tile([32, 1], F32)
            nc.sync.dma_start(out=b1c_sb[:], in_=ct["b1c"][:, :])
            dW2A_sb = sing.tile([32, 32], F32)
            nc.sync.dma_start(out=dW2A_sb[:], in_=ct["dW2A"][:, :])
            cW2B_sb = sing.tile([32, 32], F32)
            nc.sync.dma_start(out=cW2B_sb[:], in_=ct["cW2B"][:, :])
            bcomb_sb = sing.tile([32, 1], F32)
            nc.sync.dma_start(out=bcomb_sb[:], in_=ct["b_comb"][:, :])
            fusg_sb = sing.tile([32, 1], F32)
            nc.sync.dma_start(out=fusg_sb[:], in_=ct["fusg"][:, :])
            fusb_sb = sing.tile([32, 1], F32)
            nc.sync.dma_start(out=fusb_sb[:], in_=ct["fusb"][:, :])
            ones32_sb = sing.tile([32, 1], F32)
            nc.sync.dma_start(out=ones32_sb[:], in_=ct["ones32"][:, :])
            ones1x32_sb = sing.tile([1, 32], F32)
            nc.sync.dma_start(out=ones1x32_sb[:], in_=ct["ones1x32"][:, :])

            eps_ln = sing.tile([P, 1], F32)
            nc.vector.memset(eps_ln[:], LN_EPS)
            eps_f1 = sing.tile([1, 1], F32)
            nc.vector.memset(eps_f1[:], LN_EPS)

            # DRAM scratch
            hT_d = dram.tile([32, EP], F32)
            XSl_d = dram.tile([NLP, HID], F32)
            XSf_d = dram.tile([NG, HID], F32)

            AF = mybir.ActivationFunctionType
            OP = mybir.AluOpType

            import os as _os
            PHASES = int(_os.environ.get("KPHASES", "3"))

            # ================= Phase 1: edge encoder (feat-major) =================
            FE = 256
            nchunk = (EP + FE - 1) // FE if PHASES >= 1 else 0
            for ci in range(nchunk):
                w = min(FE, EP - ci * FE)
                s0 = ci * FE
                # h1 = relu(d (x) dW1 + b1)
                p1 = ps_ef.tile([32, FE], F32, tag="ef")
                dTt = encp.tile([1, FE], F32, tag="dTt")
                nc.sync.dma_start(out=dTt[:, :w], in_=dT[:, s0:s0 + w])
                cTt = encp.tile([1, FE], F32, tag="cTt")
                nc.sync.dma_start(out=cTt[:, :w], in_=cT[:, s0:s0 + w])
                relt = encp.tile([32, FE], F32, tag="relt")
                nc.sync.dma_start(out=relt[:, :w], in_=relT[:, s0:s0 + w])

                nc.tensor.matmul(p1[:, :w], lhsT=dW1_sb[:], rhs=dTt[:, :w],
                                 start=True, stop=True)
                h1d = encp.tile([32, FE], F32, tag="h1d")
                nc.scalar.activation(h1d[:, :w], p1[:, :w], AF.Relu,
                                     bias=b1d_sb[:], scale=1.0)
                p1b = ps_ef.tile([32, FE], F32, tag="ef")
                nc.tensor.matmul(p1b[:, :w], lhsT=cW1_sb[:], rhs=cTt[:, :w],
                                 start=True, stop=True)
                h1c = encp.tile([32, FE], F32, tag="h1c")
                nc.scalar.activation(h1c[:, :w], p1b[:, :w], AF.Relu,
                                     bias=b1c_sb[:], scale=1.0)
                # hpre = dW2A^T h1d + cW2B^T h1c  (+ rel + b_comb)
                p2 = ps_ef.tile([32, FE], F32, tag="ef")
                nc.tensor.matmul(p2[:, :w], lhsT=dW2A_sb[:], rhs=h1d[:, :w],
                                 start=True, stop=False)
                nc.tensor.matmul(p2[:, :w], lhsT=cW2B_sb[:], rhs=h1c[:, :w],
                                 start=False, stop=True)
                hpre = encp.tile([32, FE], F32, tag="hpre")
                nc.vector.tensor_add(hpre[:, :w], p2[:, :w], relt[:, :w])
                nc.scalar.activation(hpre[:, :w], hpre[:, :w], AF.Identity,
                                     bias=bcomb_sb[:], scale=1.0)
                # LN over the 32 partitions (PE column-sum trick)
                sq = encp.tile([32, FE], F32, tag="sq")
                nc.vector.tensor_mul(sq[:, :w], hpre[:, :w], hpre[:, :w])
                pm = ps_m.tile([64, FE], F32, tag="m")
                nc.tensor.matmul(pm[0:1, :w], lhsT=ones32_sb[:], rhs=hpre[:, :w],
                                 start=True, stop=True)
                nc.tensor.matmul(pm[32:33, :w], lhsT=ones32_sb[:], rhs=sq[:, :w],
                                 start=True, stop=True)
                stats = encp.tile([1, 2, FE], F32, tag="stats")
                nc.scalar.activation(stats[:, 0, :w], pm[0:1, :w], AF.Copy,
                                     scale=1.0 / 32.0)          # mean
                nc.scalar.activation(stats[:, 1, :w], pm[32:33, :w], AF.Copy,
                                     scale=1.0 / 32.0)          # E[x^2]
                var = encp.tile([1, FE], F32, tag="var")
                nc.vector.tensor_mul(var[:, :w], stats[:, 0, :w], stats[:, 0, :w])
                nc.vector.tensor_tensor(out=var[:, :w], in0=stats[:, 1, :w],
                                        in1=var[:, :w], op=OP.subtract)
                sd = encp.tile([1, FE], F32, tag="sd")
                nc.scalar.activation(sd[:, :w], var[:, :w], AF.Sqrt,
                                     bias=eps_f1[0:1, :], scale=1.0)
                rs = encp.tile([1, FE], F32, tag="rs")
                nc.vector.reciprocal(rs[:, :w], sd[:, :w])
                # broadcast mean/rs to 32 partitions via PE outer product
                pmb = ps_m.tile([64, FE], F32, tag="m")
                nc.tensor.matmul(pmb[0:32, :w], lhsT=ones1x32_sb[:],
                                 rhs=stats[:, 0, :w], start=True, stop=True)
                nc.tensor.matmul(pmb[32:64, :w], lhsT=ones1x32_sb[:],
                                 rhs=rs[:, :w], start=True, stop=True)
                hn = encp.tile([32, FE], F32, tag="hn")
                nc.vector.tensor_tensor(out=hn[:, :w], in0=hpre[:, :w],
                                        in1=pmb[0:32, :w], op=OP.subtract)
                nc.vector.tensor_mul(hn[:, :w], hn[:, :w], pmb[32:64, :w])
                nc.vector.tensor_scalar(out=hn[:, :w], in0=hn[:, :w],
                                        scalar1=fusg_sb[:], scalar2=fusb_sb[:],
                                        op0=OP.mult, op1=OP.add)
                hfin = encp.tile([32, FE], F32, tag="hfin")
                nc.scalar.activation(hfin[:, :w], hn[:, :w], AF.Relu)
                nc.sync.dma_start(out=hT_d[:, s0:s0 + w], in_=hfin[:, :w])

            # ================= Phase 2: input LN + projection =================
            for b in range(NB if PHASES >= 2 else 0):
                xt = work.tile([P, IN], F32, tag="xt")
                nc.sync.dma_start(out=xt[:], in_=x_in[b * P:(b + 1) * P, :])
                st6 = wsm.tile([P, 6], F32, tag="st6")
                nc.vector.bn_stats(out=st6[:], in_=xt[:])
                mv = wsm.tile([P, 2], F32, tag="mv")
                nc.vector.bn_aggr(out=mv[:], in_=st6[:])
                sd2 = wsm.tile([P, 1], F32, tag="sd2")
                nc.scalar.activation(sd2[:], mv[:, 1:2], AF.Sqrt,
                                     bias=eps_ln[:], scale=1.0)
                rs2 = wsm.tile([P, 1], F32, tag="rs2")
                nc.vector.reciprocal(rs2[:], sd2[:])
                xn = work.tile([P, IN], F32, tag="xn")
                nc.vector.tensor_scalar(out=xn[:], in0=xt[:],
                                        scalar1=mv[:, 0:1], scalar2=rs2[:],
                                        op0=OP.subtract, op1=OP.mult)
                nc.vector.tensor_mul(xn[:], xn[:], in_g_bc[:])
                nc.vector.tensor_add(xn[:], xn[:], in_b_bc[:])
                # transpose halves -> lhsT
                ptA = ps_tr.tile([P, P], F32, tag="m")
                nc.tensor.transpose(ptA[:], xn[:, 0:P], identity[:])
                tA = work.tile([P, P], F32, tag="tA")
                nc.scalar.copy(tA[:], ptA[:])
                ptB = ps_tr.tile([P, P], F32, tag="m")
                nc.tensor.transpose(ptB[:], xn[:, P:IN], identity[:])
                tB = work.tile([P, P], F32, tag="tB")
                nc.scalar.copy(tB[:], ptB[:])
                px = ps_m.tile([P, HID], F32, tag="m")
                nc.tensor.matmul(px[:], lhsT=tA[:], rhs=projW_lo[:],
                                 start=True, stop=False)
                nc.tensor.matmul(px[:], lhsT=tB[:], rhs=projW_hi[:],
                                 start=False, stop=True)
                x0 = work.tile([P, HID], F32, tag="x0")
                nc.vector.tensor_add(x0[:], px[:], proj_b_bc[:])
                nc.scalar.copy(xnm_sb[:, b, :], x0[:])
                ptx = ps_tr.tile([P, P], F32, tag="m")
                nc.tensor.transpose(ptx[:], x0[:], identity[:])
                nc.scalar.copy(xT_sb[:, b * P:(b + 1) * P], ptx[:])

            if KDEBUG:
                nc.sync.dma_start(out=dbg_h[:, :], in_=hT_d[:, :])
                nc.sync.dma_start(out=dbg_x0[:, :, :], in_=xnm_sb[:, :, :])

            # ================= Phase 3: GAT layers =================
            for i in range(L if PHASES >= 3 else 0):
                last = i == L - 1
                # --- XS / XD' production ---
                for b in range(NB):
                    pxs = ps_m.tile([P, HID], F32, tag="m")
                    nc.tensor.matmul(pxs[:], lhsT=xT_sb[:, b * P:(b + 1) * P],
                                     rhs=Wsrc_sb[i][:], start=True, stop=True)
                    xs_st = work.tile([P, HID], F32, tag="xs_st")
                    nc.scalar.copy(xs_st[:], pxs[:])
                    nc.sync.dma_start(out=XSl_d[b * P:(b + 1) * P, :],
                                      in_=xs_st[:])
                    pxd = ps_ef.tile([P, HID], F32, tag="ef")
                    nc.tensor.matmul(pxd[:], lhsT=xT_sb[:, b * P:(b + 1) * P],
                                     rhs=Wdst_sb[i][:], start=True, stop=True)
                    nc.vector.tensor_add(XD_sb[:, b, :], pxd[:], bef_bc[i][:])

                nc.gpsimd.collective_compute(
                    "AllGather", OP.bypass,
                    replica_groups=[list(range(NCORES))],
                    ins=[XSl_d[:, :].opt()], outs=[XSf_d[:, :].opt()])
                if KDEBUG and i == 0:
                    nc.sync.dma_start(out=dbg_xs[:, :], in_=XSf_d[:, :])
                    nc.sync.dma_start(out=dbg_xd[:, :, :], in_=XD_sb[:, :, :])

                # --- edge pass ---
                for b in range(NB):
                    acc = ps_acc.tile([P, HID], F32, tag="acc")
                    accd = ps_den.tile([P, H], F32, tag="accd")
                    hTb = work.tile([32, 18 * P], F32, tag="hTb")
                    nc.sync.dma_start(
                        out=hTb[:, :T_b[b] * P],
                        in_=hT_d[:, tile_base[b] * P:(tile_base[b] + T_b[b]) * P])
                    for t in range(T_b[b]):
                        ti = tile_base[b] + t
                        first, lastt = t == 0, t == T_b[b] - 1
                        # gather xs rows
                        xs_t = work.tile([P, HID], F32, tag="xs_t")
                        nc.gpsimd.indirect_dma_start(
                            out=xs_t[:], out_offset=None,
                            in_=XSf_d[:, :],
                            in_offset=bass.IndirectOffsetOnAxis(
                                ap=srcg_sb[:, ti:ti + 1], axis=0))
                        # one-hot and transposed one-hot
                        oh = work.tile([P, P], F32, tag="oh")
                        nc.vector.tensor_tensor(
                            out=oh[:],
                            in0=dstf_sb[:, ti:ti + 1].to_broadcast([P, P]),
                            in1=iota_row[:], op=OP.is_equal)
                        ptr = ps_tr.tile([P, P], F32, tag="m")
                        nc.tensor.transpose(ptr[:], oh[:], identity[:])
                        ohT = work.tile([P, P], F32, tag="ohT")
                        nc.scalar.copy(ohT[:], ptr[:])
                        # ef + xd -> psum
                        pef = ps_ef.tile([P, HID], F32, tag="ef")
                        nc.tensor.matmul(pef[:], lhsT=hTb[:, t * P:(t + 1) * P],
                                         rhs=Wfold_sb[i][:],
                                         start=True, stop=False)
                        nc.tensor.matmul(pef[:], lhsT=ohT[:], rhs=XD_sb[:, b, :],
                                         start=False, stop=True)
                        # pre-activation, tanh
                        pre = work.tile([P, HID], F32, tag="pre")
                        nc.vector.tensor_add(pre[:], xs_t[:], pef[:])
                        a_t = work.tile([P, HID], F32, tag="a_t")
                        nc.scalar.activation(a_t[:], pre[:], AF.Tanh)
                        # alpha = sum_c a*att
                        wa = work.tile([P, H, C], F32, tag="wa")
                        nc.vector.tensor_mul(
                            wa[:].rearrange("p h c -> p (h c)"), a_t[:],
                            att_bc[i][:])
                        alpha = wsm.tile([P, H], F32, tag="alpha")
                        nc.vector.reduce_sum(out=alpha[:], in_=wa[:, :, :],
                                             axis=mybir.AxisListType.X)
                        if not last:
                            dec = wsm.tile([P, H], F32, tag="dec")
                            nc.vector.tensor_scalar(
                                out=dec[:], in0=ss_bc[i][:],
                                scalar1=logd_sb[:, ti:ti + 1], scalar2=None,
                                op0=OP.mult)
                            nc.scalar.activation(dec[:], dec[:], AF.Exp)
                            nc.vector.tensor_mul(alpha[:], alpha[:], dec[:])
                        ea = wsm.tile([P, H], F32, tag="ea")
                        nc.scalar.activation(ea[:], alpha[:], AF.Exp)
                        # msg = xs * ea (per-head broadcast)
                        msg = work.tile([P, H, C], F32, tag="msg")
                        nc.vector.tensor_tensor(
                            out=msg[:, :, :],
                            in0=xs_t[:].rearrange("p (h c) -> p h c", h=H),
                            in1=ea[:].unsqueeze(2).to_broadcast([P, H, C]),
                            op=OP.mult)
                        # segment accumulate
                        nc.tensor.matmul(acc[:], lhsT=oh[:],
                                         rhs=msg[:].rearrange("p h c -> p (h c)"),
                                         start=first, stop=lastt)
                        nc.tensor.matmul(accd[:], lhsT=oh[:],
                                         rhs=ea[:], start=first, stop=lastt)

                    # --- block epilogue ---
                    den = wsm.tile([P, H], F32, tag="den")
                    nc.vector.tensor_scalar(out=den[:], in0=accd[:],
                                            scalar1=1e-8, scalar2=None,
                                            op0=OP.add)
                    rec = wsm.tile([P, H], F32, tag="rec")
                    nc.vector.reciprocal(rec[:], den[:])
                    o1 = work.tile([P, H, C], F32, tag="o1")
                    nc.vector.tensor_tensor(
                        out=o1[:, :, :],
                        in0=acc[:].rearrange("p (h c) -> p h c", h=H),
                        in1=rec[:].unsqueeze(2).to_broadcast([P, H, C]),
                        op=OP.mult)
                    o1f = o1[:].rearrange("p h c -> p (h c)")
                    st6b = wsm.tile([P, 6], F32, tag="st6b")
                    nc.vector.bn_stats(out=st6b[:], in_=o1f)
                    mvb = wsm.tile([P, 2], F32, tag="mvb")
                    nc.vector.bn_aggr(out=mvb[:], in_=st6b[:])
                    sdb = wsm.tile([P, 1], F32, tag="sdb")
                    nc.scalar.activation(sdb[:], mvb[:, 1:2], AF.Sqrt,
                                         bias=eps_ln[:], scale=1.0)
                    rsb = wsm.tile([P, 1], F32, tag="rsb")
                    nc.vector.reciprocal(rsb[:], sdb[:])
                    xn2 = work.tile([P, HID], F32, tag="xn2")
                    nc.vector.tensor_scalar(out=xn2[:], in0=o1f,
                                            scalar1=mvb[:, 0:1], scalar2=rsb[:],
                                            op0=OP.subtract, op1=OP.mult)
                    nc.vector.tensor_mul(xn2[:], xn2[:], ng_bc[i][:])
                    nc.vector.tensor_add(xn2[:], xn2[:], nb_bc[i][:])
                    # elu(x) = relu(x) + exp(min(x,0)) - 1
                    tneg = work.tile([P, HID], F32, tag="tneg")
                    nc.vector.tensor_scalar(out=tneg[:], in0=xn2[:],
                                            scalar1=0.0, scalar2=None,
                                            op0=OP.min)
                    e1 = work.tile([P, HID], F32, tag="e1")
                    nc.scalar.activation(e1[:], tneg[:], AF.Exp)
                    r1 = work.tile([P, HID], F32, tag="r1")
                    nc.scalar.activation(r1[:], xn2[:], AF.Relu)
                    xout = work.tile([P, HID], F32, tag="xout")
                    nc.vector.tensor_add(xout[:], r1[:], e1[:])
                    nc.vector.tensor_scalar(out=xout[:], in0=xout[:],
                                            scalar1=-1.0, scalar2=None,
                                            op0=OP.add)
                    nc.vector.tensor_add(xout[:], xout[:], xnm_sb[:, b, :])
                    if last:
                        nc.sync.dma_start(out=out[b * P:(b + 1) * P, :],
                                          in_=xout[:])
                    else:
                        nc.scalar.copy(xnm_sb[:, b, :], xout[:])
                        ptx2 = ps_tr.tile([P, P], F32, tag="m")
                        nc.tensor.transpose(ptx2[:], xout[:], identity[:])
                        nc.scalar.copy(xT_sb[:, b * P:(b + 1) * P], ptx2[:])
                        if KDEBUG and i == 0:
                            nc.sync.dma_start(out=dbg_x1[:, b, :], in_=xout[:])

    _split_waits(nc)
    return nc


# ---------------- public entry point ----------------
def kernel(**inputs):
    _install_ntff_hook()
    per_core, consts, meta = _prep(inputs)
    nc = _build(consts, meta)
    in_maps = [per_core[k] for k in range(NCORES)]
    res = run_bass_kernel_spmd(nc, in_maps, list(range(NCORES)),
                               trace=bool(int(__import__("os").environ.get(
                                   "KERNEL_TRACE", "0"))))
    kernel.last_exec_time_ns = res.exec_time_ns
    kernel.last_results = res
    outs = [res.results[k]["out"][:NL] for k in range(NCORES)]
    return np.concatenate(outs, axis=0).astype(np.float32)


kernel.last_exec_time_ns = None


# revision 17
# speedup vs baseline: 1.2939x; 1.2939x over previous
"""PlantCCCEncoder (3-layer GAT over 50k nodes / 800k edges) on 8 trn2 cores.

Strategy: edges sharded by dst node range (each core owns 6250 dst nodes and
all edges pointing into them -> segment softmax/sum are core-local, no
all-reduce). Node features replicated via a per-layer AllGather of the
src-projection table XS. Per-edge xs rows fetched with indirect DMA gathers;
xd and edge-feature terms come from PE matmuls (one-hot expansion) off
SBUF-resident tables, overlapped under the gather.

Self-contained: hardcodes all shapes; builds + runs the Bass program via
run_bass_kernel_spmd on cores 0-7.
"""
import sys
import types

sys.path.insert(0, "/opt/trn_rl_repo")

import numpy as np

import concourse.bass as bass
import concourse.tile as tile
from concourse import mybir
from concourse.bass_utils import run_bass_kernel_spmd

# ---------------- model dims (from the reference) ----------------
N = 50000
E = 800000
IN = 256
HID = 128
H = 4
C = 32
L = 3
REL = 100
RD = 16
EH = 32
LN_EPS = 1e-5

NCORES = 8
P = 128
NL = N // NCORES            # 6250 local dst nodes per core
NB = (NL + P - 1) // P      # 49 dst blocks
NLP = NB * P                # 6272 padded local nodes
NG = NCORES * NLP           # 50176 rows in the all-gathered XS table

F32 = mybir.dt.float32
F16 = mybir.dt.float16
I32 = mybir.dt.int32

PAD_COL = 200.0             # one-hot column id for padding edges (never matches)


# ---------------- NTFF profile hook (for trace timing) ----------------
def _install_ntff_hook():
    try:
        import antenv  # noqa: F401
        if "antenv.axon_hooks" in sys.modules:
            return
        mod = types.ModuleType("antenv.axon_hooks")
        _hook = [None]
        mod.set_axon_ntff_profile_hook = lambda h: _hook.__setitem__(0, h)
        mod.get_axon_ntff_profile_hook = lambda: _hook[0]
        sys.modules["antenv.axon_hooks"] = mod
        setattr(sys.modules["antenv"], "axon_hooks", mod)
        from trn_agent_boot.trn_boot import _ntff_profile_via_ctypes
        mod.set_axon_ntff_profile_hook(
            _ntff_profile_via_ctypes("/opt/axon/libaxon_pjrt.so"))
    except Exception:
        pass


# ---------------- walrus 1-wait-per-instruction workaround ----------------
def _split_waits(nc):
    def _get_si(ins):
        si = ins.sync_info
        return si() if callable(si) else si

    for f in nc.m.functions:
        for bb in f.blocks:
            insts = list(bb.instructions)
            out = []
            changed = False
            for ins in insts:
                if not isinstance(ins, mybir.InstEventSemaphore):
                    si = _get_si(ins)
                    if si is not None and len(si.on_wait) > 1:
                        waits = list(si.on_wait)
                        for k, w in enumerate(waits[1:]):
                            ev = mybir.InstEventSemaphore(
                                name=f"{ins.name}-ws{k}", engine=ins.engine,
                                ins=[], outs=[],
                                sync_info=mybir.SyncInfo(on_wait=[w], on_update=[]))
                            ev.bass_nofuse = True
                            out.append(ev)
                        ins.sync_info = mybir.SyncInfo(
                            on_wait=waits[:1], on_update=list(si.on_update))
                        changed = True
                out.append(ins)
            if changed:
                bb.instructions = out


# ---------------- host-side preprocessing ----------------
def _prep(inputs):
    x = np.asarray(inputs["x"], np.float32)
    edge_index = np.asarray(inputs["edge_index"]).astype(np.int64)
    edge_attr = np.asarray(inputs["edge_attr"], np.float32)

    g = {k: np.asarray(v, np.float32) for k, v in inputs.items()
         if k not in ("x", "edge_index", "edge_attr")}

    src = edge_index[0]
    dst = edge_index[1]

    # ---- parameter folding (host, parameters only) ----
    fus_W1 = g["fus_W1"]                     # [48, 32]
    A, B, Cc = fus_W1[:RD], fus_W1[RD:2 * RD], fus_W1[2 * RD:]
    dW2A = g["dist_W2"] @ A                  # [32, 32]
    cW2B = g["coexp_W2"] @ B                 # [32, 32]
    relp = g["rel_emb"] @ Cc                 # [100, 32]
    b_comb = (g["fus_b1"] + g["dist_b2"] @ A + g["coexp_b2"] @ B)  # [32]
    Wfold = np.stack([g["fus_W2"] @ g["Wedge"][i] for i in range(L)])      # [3,32,128]
    bias_ef = np.stack([g["fus_b2"] @ g["Wedge"][i] for i in range(L)])    # [3,128]

    # ---- edge sharding: sort by dst, shard by dst range, block by 128 ----
    order = np.argsort(dst, kind="stable")
    src_s, dst_s = src[order], dst[order]
    ea_s = edge_attr[order]

    core_of = dst_s // NL
    dl = dst_s % NL
    blk = dl // P
    col = dl % P

    # per (core, block) edge lists
    counts = np.zeros((NCORES, NB), np.int64)
    for k in range(NCORES):
        m = core_of == k
        bb = blk[m]
        cnt = np.bincount(bb, minlength=NB)
        counts[k] = cnt
    T_b = np.maximum(1, ((counts.max(axis=0) + P - 1) // P)).astype(np.int64)  # [NB]
    TT = int(T_b.sum())
    EP = TT * P
    tile_base = np.concatenate([[0], np.cumsum(T_b)])[:-1]  # first tile idx per block

    # slot assignment per core
    rng_rows = np.arange(N, dtype=np.int64)
    pad_row = (rng_rows // NL) * NLP + (rng_rows % NL)      # global node -> XS row

    per_core = []
    for k in range(NCORES):
        m = core_of == k
        e_src = src_s[m]
        e_col = col[m]
        e_blk = blk[m]
        e_d = ea_s[m, 0]
        e_c = ea_s[m, 1]
        e_rel = np.clip(ea_s[m, 2].astype(np.int64), 0, REL - 1)

        srcg = np.zeros(EP, np.int32)
        dstf = np.full(EP, PAD_COL, np.float32)
        logd = np.zeros(EP, np.float32)
        dT = np.zeros(EP, np.float32)
        cT = np.zeros(EP, np.float32)
        relT = np.zeros((32, EP), np.float32)

        # stable order within block
        ob = np.argsort(e_blk, kind="stable")
        e_src, e_col, e_blk = e_src[ob], e_col[ob], e_blk[ob]
        e_d, e_c, e_rel = e_d[ob], e_c[ob], e_rel[ob]
        # slot = (tile_base[b]*128 + position within block's slots)
        start = 0
        for b in range(NB):
            nb_ = counts[k, b]
            sl0 = tile_base[b] * P
            sl = np.arange(nb_) + sl0
            seg = slice(start, start + nb_)
            srcg[sl] = pad_row[e_src[seg]]
            dstf[sl] = e_col[seg]
            logd[sl] = np.log(e_d[seg])
            dT[sl] = e_d[seg]
            cT[sl] = e_c[seg]
            relT[:, sl] = relp[e_rel[seg]].T
            start += nb_

        # [128, TT] layouts: slot = ti*128 + p  ->  [p, ti]
        srcg2 = srcg.reshape(TT, P).T.copy()
        dstf2 = dstf.reshape(TT, P).T.copy()
        logd2 = logd.reshape(TT, P).T.copy()

        xk = np.zeros((NLP, IN), np.float32)
        xk[:NL] = x[k * NL:(k + 1) * NL]

        per_core.append(dict(
            x_in=xk, srcg=srcg2, dstf=dstf2, logd=logd2,
            dT=dT[None, :].astype(np.float16), cT=cT[None, :].astype(np.float16),
            relT=relT,
        ))

    def rep(row):
        return np.tile(np.asarray(row, np.float32)[None, :], (P, 1))

    consts = dict(
        identity=np.eye(P, dtype=np.float32),
        identity16=np.eye(P, dtype=np.float16),
        iota_row=np.tile(np.arange(P, dtype=np.float32), (P, 1)),
        ones32=np.ones((32, 1), np.float32),
        ones1x32=np.ones((1, 32), np.float32),
        in_g=rep(g["in_g"]), in_b=rep(g["in_b"]),
        proj_W=g["proj_W"], proj_b=rep(g["proj_b"]),
        dW1=g["dist_W1"].astype(np.float16), cW1=g["coexp_W1"].astype(np.float16),
        b1d=g["dist_b1"][:, None], b1c=g["coexp_b1"][:, None],
        dW2A=dW2A.astype(np.float16), cW2B=cW2B.astype(np.float16), b_comb=b_comb[:, None],
        fusg=g["fus_ln_g"][:, None], fusb=g["fus_ln_b"][:, None],
        Wsrc=g["Wsrc"], Wdst=g["Wdst"], Wfold=Wfold.astype(np.float16),
        bias_ef=np.stack([rep(bias_ef[i]) for i in range(L)]),
        att=np.stack([rep(g["att"][i].reshape(H * C)) for i in range(L)]),
        sscale=np.stack([rep(g["sscale"][i]) for i in range(L)]),
        ng=np.stack([rep(g["ng"][i]) for i in range(L)]),
        nb=np.stack([rep(g["nb"][i]) for i in range(L)]),
    )
    meta = dict(T_b=T_b.tolist(), tile_base=tile_base.tolist(), TT=TT, EP=EP)
    return per_core, consts, meta


# ---------------- device program ----------------
def _build(consts, meta):
    T_b, tile_base, TT, EP = (meta["T_b"], meta["tile_base"],
                              meta["TT"], meta["EP"])
    nc = bass.Bass(num_devices=NCORES)

    # per-core inputs
    x_in = nc.declare_dram_parameter("x_in", [NLP, IN], F32, isOutput=False)
    srcg = nc.declare_dram_parameter("srcg", [P, TT], I32, isOutput=False)
    dstf = nc.declare_dram_parameter("dstf", [P, TT], F32, isOutput=False)
    logd = nc.declare_dram_parameter("logd", [P, TT], F32, isOutput=False)
    dT = nc.declare_dram_parameter("dT", [1, EP], F16, isOutput=False)
    cT = nc.declare_dram_parameter("cT", [1, EP], F16, isOutput=False)
    relT = nc.declare_dram_parameter("relT", [32, EP], F32, isOutput=False)
    out = nc.declare_dram_parameter("out", [NLP, HID], F32, isOutput=True)
    import os as _os
    KDEBUG = bool(int(_os.environ.get("KDEBUG", "0")))
    if KDEBUG:
        dbg_h = nc.declare_dram_parameter("dbg_h", [32, EP], F32, isOutput=True)
        dbg_x0 = nc.declare_dram_parameter("dbg_x0", [P, NB, HID], F32, isOutput=True)
        dbg_xs = nc.declare_dram_parameter("dbg_xs", [NG, HID], F32, isOutput=True)
        dbg_xd = nc.declare_dram_parameter("dbg_xd", [P, NB, HID], F32, isOutput=True)
        dbg_x1 = nc.declare_dram_parameter("dbg_x1", [P, NB, HID], F32, isOutput=True)

    ct = {k: nc.inline_tensor(np.ascontiguousarray(v), name=f"c_{k}")
          for k, v in consts.items()}

    with tile.TileContext(nc) as tc:
        import contextlib
        ctx = contextlib.ExitStack()
        with ctx:
            sing = ctx.enter_context(tc.tile_pool(name="sing", bufs=1))
            work = ctx.enter_context(tc.tile_pool(name="work", bufs=3))
            wsm = ctx.enter_context(tc.tile_pool(name="wsm", bufs=3))
            ps_acc = ctx.enter_context(tc.tile_pool(name="psacc", bufs=2, space="PSUM"))
            ps_den = ctx.enter_context(tc.tile_pool(name="psden", bufs=2, space="PSUM"))
            ps_ef = ctx.enter_context(tc.tile_pool(name="psef", bufs=2, space="PSUM"))
            ps_m = ctx.enter_context(tc.tile_pool(name="psm", bufs=2, space="PSUM"))
            ps_tr = ps_m
            dram = ctx.enter_context(tc.tile_pool(name="dram", bufs=1, space="DRAM"))
            encp = ctx.enter_context(tc.tile_pool(name="encp", bufs=2))

            _uid = [0]

            def bc_load(mat_t, parts, width, tag=None):
                """Load a pre-replicated [parts, width] constant."""
                _uid[0] += 1
                t = sing.tile([parts, width], F32, tag=tag or f"bc{_uid[0]}")
                nc.sync.dma_start(out=t[:], in_=mat_t[:, :])
                return t

            # ---- persistent SBUF tiles ----
            identity = sing.tile([P, P], F32)
            nc.sync.dma_start(out=identity[:], in_=ct["identity"][:, :])
            identity16 = sing.tile([P, P], F16, tag="id16")
            nc.sync.dma_start(out=identity16[:], in_=ct["identity16"][:, :])
            iota_row = sing.tile([P, P], F32)
            nc.sync.dma_start(out=iota_row[:], in_=ct["iota_row"][:, :])
            srcg_sb = sing.tile([P, TT], I32)
            nc.sync.dma_start(out=srcg_sb[:], in_=srcg[:, :])
            dstf_sb = sing.tile([P, TT], F32)
            nc.sync.dma_start(out=dstf_sb[:], in_=dstf[:, :])
            logd_sb = sing.tile([P, TT], F32)
            nc.sync.dma_start(out=logd_sb[:], in_=logd[:, :])

            xT_sb = sing.tile([P, NLP], F32)          # x feat-major (lhsT source)
            xnm_sb = sing.tile([P, NB, HID], F32)     # x node-major  [:, b, :]
            XD_sb = sing.tile([P, NB, HID], F16)      # XD' node-major

            in_g_bc = bc_load(ct["in_g"], P, IN)
            in_b_bc = bc_load(ct["in_b"], P, IN)
            proj_b_bc = bc_load(ct["proj_b"], P, HID)
            projW_lo = sing.tile([P, HID], F32)
            nc.sync.dma_start(out=projW_lo[:], in_=ct["proj_W"][0:P, :])
            projW_hi = sing.tile([P, HID], F32)
            nc.sync.dma_start(out=projW_hi[:], in_=ct["proj_W"][P:IN, :])

            Wsrc_sb, Wdst_sb, Wfold_sb = [], [], []
            att_bc, ss_bc, ng_bc, nb_bc, bef_bc = [], [], [], [], []
            for i in range(L):
                t = sing.tile([HID, HID], F32, tag=f"wsrc{i}")
                nc.sync.dma_start(out=t[:], in_=ct["Wsrc"][i, :, :])
                Wsrc_sb.append(t)
                t = sing.tile([HID, HID], F32, tag=f"wdst{i}")
                nc.sync.dma_start(out=t[:], in_=ct["Wdst"][i, :, :])
                Wdst_sb.append(t)
                t = sing.tile([32, HID], F16, tag=f"wfold{i}")
                nc.sync.dma_start(out=t[:], in_=ct["Wfold"][i, :, :])
                Wfold_sb.append(t)
                att_bc.append(bc_load(ct["att"][i], P, H * C))
                ss_bc.append(bc_load(ct["sscale"][i], P, H))
                ng_bc.append(bc_load(ct["ng"][i], P, HID))
                nb_bc.append(bc_load(ct["nb"][i], P, HID))
                bef_bc.append(bc_load(ct["bias_ef"][i], P, HID))

            # encoder consts (feat-major; per-partition columns)
            dW1_sb = sing.tile([1, 32], F16)
            nc.sync.dma_start(out=dW1_sb[:], in_=ct["dW1"][:, :])
            cW1_sb = sing.tile([1, 32], F16)
            nc.sync.dma_start(out=cW1_sb[:], in_=ct["cW1"][:, :])
            b1d_sb = sing.tile([32, 1], F32)
            nc.sync.dma_start(out=b1d_sb[:], in_=ct["b1d"][:, :])
            b1c_sb = sing.tile([32, 1], F32)
            nc.sync.dma_start(out=b1c_sb[:], in_=ct["b1c"][:, :])
            dW2A_sb = sing.tile([32, 32], F16)
            nc.sync.dma_start(out=dW2A_sb[:], in_=ct["dW2A"][:, :])
            cW2B_sb = sing.tile([32, 32], F16)
            nc.sync.dma_start(out=cW2B_sb[:], in_=ct["cW2B"][:, :])
            bcomb_sb = sing.tile([32, 1], F32)
            nc.sync.dma_start(out=bcomb_sb[:], in_=ct["b_comb"][:, :])
            fusg_sb = sing.tile([32, 1], F32)
            nc.sync.dma_start(out=fusg_sb[:], in_=ct["fusg"][:, :])
            fusb_sb = sing.tile([32, 1], F32)
            nc.sync.dma_start(out=fusb_sb[:], in_=ct["fusb"][:, :])
            ones32_sb = sing.tile([32, 1], F32)
            nc.sync.dma_start(out=ones32_sb[:], in_=ct["ones32"][:, :])
            ones1x32_sb = sing.tile([1, 32], F32)
            nc.sync.dma_start(out=ones1x32_sb[:], in_=ct["ones1x32"][:, :])

            eps_ln = sing.tile([P, 1], F32)
            nc.vector.memset(eps_ln[:], LN_EPS)
            eps_f1 = sing.tile([1, 1], F32)
            nc.vector.memset(eps_f1[:], LN_EPS)

            # DRAM scratch
            hT_d = dram.tile([32, EP], F16)
            XSl_d = dram.tile([NLP, HID], F32)
            XSf_d = dram.tile([NG, HID], F32)

            AF = mybir.ActivationFunctionType
            OP = mybir.AluOpType

            import os as _os
            PHASES = int(_os.environ.get("KPHASES", "3"))

            # ================= Phase 1: edge encoder (feat-major) =================
            FE = 256
            nchunk = (EP + FE - 1) // FE if PHASES >= 1 else 0
            for ci in range(nchunk):
                w = min(FE, EP - ci * FE)
                s0 = ci * FE
                # h1 = relu(d (x) dW1 + b1)
                p1 = ps_ef.tile([32, FE], F32, tag="ef")
                dTt = encp.tile([1, FE], F16, tag="dTt")
                nc.sync.dma_start(out=dTt[:, :w], in_=dT[:, s0:s0 + w])
                cTt = encp.tile([1, FE], F16, tag="cTt")
                nc.sync.dma_start(out=cTt[:, :w], in_=cT[:, s0:s0 + w])
                relt = encp.tile([32, FE], F32, tag="relt")
                nc.sync.dma_start(out=relt[:, :w], in_=relT[:, s0:s0 + w])

                nc.tensor.matmul(p1[:, :w], lhsT=dW1_sb[:], rhs=dTt[:, :w],
                                 start=True, stop=True)
                h1d = encp.tile([32, FE], F16, tag="h1d")
                nc.scalar.activation(h1d[:, :w], p1[:, :w], AF.Relu,
                                     bias=b1d_sb[:], scale=1.0)
                p1b = ps_ef.tile([32, FE], F32, tag="ef")
                nc.tensor.matmul(p1b[:, :w], lhsT=cW1_sb[:], rhs=cTt[:, :w],
                                 start=True, stop=True)
                h1c = encp.tile([32, FE], F16, tag="h1c")
                nc.scalar.activation(h1c[:, :w], p1b[:, :w], AF.Relu,
                                     bias=b1c_sb[:], scale=1.0)
                # hpre = dW2A^T h1d + cW2B^T h1c  (+ rel + b_comb)
                p2 = ps_ef.tile([32, FE], F32, tag="ef")
                nc.tensor.matmul(p2[:, :w], lhsT=dW2A_sb[:], rhs=h1d[:, :w],
                                 start=True, stop=False)
                nc.tensor.matmul(p2[:, :w], lhsT=cW2B_sb[:], rhs=h1c[:, :w],
                                 start=False, stop=True)
                hpre = encp.tile([32, FE], F32, tag="hpre")
                nc.vector.tensor_add(hpre[:, :w], p2[:, :w], relt[:, :w])
                nc.scalar.activation(hpre[:, :w], hpre[:, :w], AF.Identity,
                                     bias=bcomb_sb[:], scale=1.0)
                # LN over the 32 partitions (PE column-sum trick)
                sq = encp.tile([32, FE], F32, tag="sq")
                nc.vector.tensor_mul(sq[:, :w], hpre[:, :w], hpre[:, :w])
                pm = ps_m.tile([64, FE], F32, tag="m")
                nc.tensor.matmul(pm[0:1, :w], lhsT=ones32_sb[:], rhs=hpre[:, :w],
                                 start=True, stop=True)
                nc.tensor.matmul(pm[32:33, :w], lhsT=ones32_sb[:], rhs=sq[:, :w],
                                 start=True, stop=True)
                stats = encp.tile([1, 2, FE], F32, tag="stats")
                nc.scalar.activation(stats[:, 0, :w], pm[0:1, :w], AF.Copy,
                                     scale=1.0 / 32.0)          # mean
                nc.scalar.activation(stats[:, 1, :w], pm[32:33, :w], AF.Copy,
                                     scale=1.0 / 32.0)          # E[x^2]
                var = encp.tile([1, FE], F32, tag="var")
                nc.vector.tensor_mul(var[:, :w], stats[:, 0, :w], stats[:, 0, :w])
                nc.vector.tensor_tensor(out=var[:, :w], in0=stats[:, 1, :w],
                                        in1=var[:, :w], op=OP.subtract)
                sd = encp.tile([1, FE], F32, tag="sd")
                nc.scalar.activation(sd[:, :w], var[:, :w], AF.Sqrt,
                                     bias=eps_f1[0:1, :], scale=1.0)
                rs = encp.tile([1, FE], F32, tag="rs")
                nc.vector.reciprocal(rs[:, :w], sd[:, :w])
                # broadcast mean/rs to 32 partitions via PE outer product
                pmb = ps_m.tile([64, FE], F32, tag="m")
                nc.tensor.matmul(pmb[0:32, :w], lhsT=ones1x32_sb[:],
                                 rhs=stats[:, 0, :w], start=True, stop=True)
                nc.tensor.matmul(pmb[32:64, :w], lhsT=ones1x32_sb[:],
                                 rhs=rs[:, :w], start=True, stop=True)
                hn = encp.tile([32, FE], F32, tag="hn")
                nc.vector.tensor_tensor(out=hn[:, :w], in0=hpre[:, :w],
                                        in1=pmb[0:32, :w], op=OP.subtract)
                nc.vector.tensor_mul(hn[:, :w], hn[:, :w], pmb[32:64, :w])
                nc.vector.tensor_scalar(out=hn[:, :w], in0=hn[:, :w],
                                        scalar1=fusg_sb[:], scalar2=fusb_sb[:],
                                        op0=OP.mult, op1=OP.add)
                hfin = encp.tile([32, FE], F16, tag="hfin")
                nc.scalar.activation(hfin[:, :w], hn[:, :w], AF.Relu)
                nc.sync.dma_start(out=hT_d[:, s0:s0 + w], in_=hfin[:, :w])

            # ================= Phase 2: input LN + projection =================
            for b in range(NB if PHASES >= 2 else 0):
                xt = work.tile([P, IN], F32, tag="xt")
                nc.sync.dma_start(out=xt[:], in_=x_in[b * P:(b + 1) * P, :])
                st6 = wsm.tile([P, 6], F32, tag="st6")
                nc.vector.bn_stats(out=st6[:], in_=xt[:])
                mv = wsm.tile([P, 2], F32, tag="mv")
                nc.vector.bn_aggr(out=mv[:], in_=st6[:])
                sd2 = wsm.tile([P, 1], F32, tag="sd2")
                nc.scalar.activation(sd2[:], mv[:, 1:2], AF.Sqrt,
                                     bias=eps_ln[:], scale=1.0)
                rs2 = wsm.tile([P, 1], F32, tag="rs2")
                nc.vector.reciprocal(rs2[:], sd2[:])
                xn = work.tile([P, IN], F32, tag="xn")
                nc.vector.tensor_scalar(out=xn[:], in0=xt[:],
                                        scalar1=mv[:, 0:1], scalar2=rs2[:],
                                        op0=OP.subtract, op1=OP.mult)
                nc.vector.tensor_mul(xn[:], xn[:], in_g_bc[:])
                nc.vector.tensor_add(xn[:], xn[:], in_b_bc[:])
                # transpose halves -> lhsT
                ptA = ps_tr.tile([P, P], F32, tag="m")
                nc.tensor.transpose(ptA[:], xn[:, 0:P], identity[:])
                tA = work.tile([P, P], F32, tag="tA")
                nc.scalar.copy(tA[:], ptA[:])
                ptB = ps_tr.tile([P, P], F32, tag="m")
                nc.tensor.transpose(ptB[:], xn[:, P:IN], identity[:])
                tB = work.tile([P, P], F32, tag="tB")
                nc.scalar.copy(tB[:], ptB[:])
                px = ps_m.tile([P, HID], F32, tag="m")
                nc.tensor.matmul(px[:], lhsT=tA[:], rhs=projW_lo[:],
                                 start=True, stop=False)
                nc.tensor.matmul(px[:], lhsT=tB[:], rhs=projW_hi[:],
                                 start=False, stop=True)
                x0 = work.tile([P, HID], F32, tag="x0")
                nc.vector.tensor_add(x0[:], px[:], proj_b_bc[:])
                nc.scalar.copy(xnm_sb[:, b, :], x0[:])
                ptx = ps_tr.tile([P, P], F32, tag="m")
                nc.tensor.transpose(ptx[:], x0[:], identity[:])
                nc.scalar.copy(xT_sb[:, b * P:(b + 1) * P], ptx[:])

            if KDEBUG:
                nc.gpsimd.dma_start(out=dbg_h[:, :], in_=hT_d[:, :])
                nc.sync.dma_start(out=dbg_x0[:, :, :], in_=xnm_sb[:, :, :])

            # ================= Phase 3: GAT layers =================
            for i in range(L if PHASES >= 3 else 0):
                last = i == L - 1
                # --- XS / XD' production ---
                for b in range(NB):
                    pxs = ps_m.tile([P, HID], F32, tag="m")
                    nc.tensor.matmul(pxs[:], lhsT=xT_sb[:, b * P:(b + 1) * P],
                                     rhs=Wsrc_sb[i][:], start=True, stop=True)
                    xs_st = work.tile([P, HID], F32, tag="xs_st")
                    nc.scalar.copy(xs_st[:], pxs[:])
                    nc.sync.dma_start(out=XSl_d[b * P:(b + 1) * P, :],
                                      in_=xs_st[:])
                    pxd = ps_ef.tile([P, HID], F32, tag="ef")
                    nc.tensor.matmul(pxd[:], lhsT=xT_sb[:, b * P:(b + 1) * P],
                                     rhs=Wdst_sb[i][:], start=True, stop=True)
                    nc.vector.tensor_add(XD_sb[:, b, :], pxd[:], bef_bc[i][:])

                nc.gpsimd.collective_compute(
                    "AllGather", OP.bypass,
                    replica_groups=[list(range(NCORES))],
                    ins=[XSl_d[:, :].opt()], outs=[XSf_d[:, :].opt()])
                if KDEBUG and i == 0:
                    nc.sync.dma_start(out=dbg_xs[:, :], in_=XSf_d[:, :])
                    nc.sync.dma_start(out=dbg_xd[:, :, :], in_=XD_sb[:, :, :])

                # --- edge pass ---
                for b in range(NB):
                    acc = ps_acc.tile([P, HID], F32, tag="acc")
                    accd = ps_den.tile([P, H], F32, tag="accd")
                    hTb = work.tile([32, 18 * P], F16, tag="hTb")
                    nc.sync.dma_start(
                        out=hTb[:, :T_b[b] * P],
                        in_=hT_d[:, tile_base[b] * P:(tile_base[b] + T_b[b]) * P])
                    for t in range(T_b[b]):
                        ti = tile_base[b] + t
                        first, lastt = t == 0, t == T_b[b] - 1
                        # gather xs rows
                        xs_t = work.tile([P, HID], F32, tag="xs_t")
                        nc.gpsimd.indirect_dma_start(
                            out=xs_t[:], out_offset=None,
                            in_=XSf_d[:, :],
                            in_offset=bass.IndirectOffsetOnAxis(
                                ap=srcg_sb[:, ti:ti + 1], axis=0))
                        # one-hot and transposed one-hot
                        oh = work.tile([P, P], F16, tag="oh")
                        nc.vector.tensor_tensor(
                            out=oh[:],
                            in0=dstf_sb[:, ti:ti + 1].to_broadcast([P, P]),
                            in1=iota_row[:], op=OP.is_equal)
                        ptr = ps_tr.tile([P, 1024], F16, tag="m")
                        nc.tensor.transpose(ptr[:, 0:P], oh[:], identity16[:])
                        ohT = work.tile([P, P], F16, tag="ohT")
                        nc.scalar.copy(ohT[:], ptr[:, 0:P])
                        # ef + xd -> psum
                        pef = ps_ef.tile([P, HID], F32, tag="ef")
                        nc.tensor.matmul(pef[:], lhsT=hTb[:, t * P:(t + 1) * P],
                                         rhs=Wfold_sb[i][:],
                                         start=True, stop=False)
                        nc.tensor.matmul(pef[:], lhsT=ohT[:], rhs=XD_sb[:, b, :],
                                         start=False, stop=True)
                        # pre-activation, tanh
                        pre = work.tile([P, HID], F32, tag="pre")
                        nc.vector.tensor_add(pre[:], xs_t[:], pef[:])
                        a_t = work.tile([P, HID], F32, tag="a_t")
                        nc.scalar.activation(a_t[:], pre[:], AF.Tanh)
                        # alpha = sum_c a*att
                        wa = work.tile([P, H, C], F32, tag="wa")
                        nc.vector.tensor_mul(
                            wa[:].rearrange("p h c -> p (h c)"), a_t[:],
                            att_bc[i][:])
                        alpha = wsm.tile([P, H], F32, tag="alpha")
                        nc.vector.reduce_sum(out=alpha[:], in_=wa[:, :, :],
                                             axis=mybir.AxisListType.X)
                        if not last:
                            dec = wsm.tile([P, H], F32, tag="dec")
                            nc.vector.tensor_scalar(
                                out=dec[:], in0=ss_bc[i][:],
                                scalar1=logd_sb[:, ti:ti + 1], scalar2=None,
                                op0=OP.mult)
                            nc.scalar.activation(dec[:], dec[:], AF.Exp)
                            nc.vector.tensor_mul(alpha[:], alpha[:], dec[:])
                        ea = wsm.tile([P, H], F16, tag="ea")
                        nc.scalar.activation(ea[:], alpha[:], AF.Exp)
                        # msg = xs * ea (per-head broadcast)
                        msg = work.tile([P, H, C], F16, tag="msg")
                        nc.vector.tensor_tensor(
                            out=msg[:, :, :],
                            in0=xs_t[:].rearrange("p (h c) -> p h c", h=H),
                            in1=ea[:].unsqueeze(2).to_broadcast([P, H, C]),
                            op=OP.mult)
                        # segment accumulate
                        nc.tensor.matmul(acc[:], lhsT=oh[:],
                                         rhs=msg[:].rearrange("p h c -> p (h c)"),
                                         start=first, stop=lastt)
                        nc.tensor.matmul(accd[:], lhsT=oh[:],
                                         rhs=ea[:], start=first, stop=lastt)

                    # --- block epilogue ---
                    den = wsm.tile([P, H], F32, tag="den")
                    nc.vector.tensor_scalar(out=den[:], in0=accd[:],
                                            scalar1=1e-8, scalar2=None,
                                            op0=OP.add)
                    rec = wsm.tile([P, H], F32, tag="rec")
                    nc.vector.reciprocal(rec[:], den[:])
                    o1 = work.tile([P, H, C], F32, tag="o1")
                    nc.vector.tensor_tensor(
                        out=o1[:, :, :],
                        in0=acc[:].rearrange("p (h c) -> p h c", h=H),
                        in1=rec[:].unsqueeze(2).to_broadcast([P, H, C]),
                        op=OP.mult)
                    o1f = o1[:].rearrange("p h c -> p (h c)")
                    st6b = wsm.tile([P, 6], F32, tag="st6b")
                    nc.vector.bn_stats(out=st6b[:], in_=o1f)
                    mvb = wsm.tile([P, 2], F32, tag="mvb")
                    nc.vector.bn_aggr(out=mvb[:], in_=st6b[:])
                    sdb = wsm.tile([P, 1], F32, tag="sdb")
                    nc.scalar.activation(sdb[:], mvb[:, 1:2], AF.Sqrt,
                                         bias=eps_ln[:], scale=1.0)
                    rsb = wsm.tile([P, 1], F32, tag="rsb")
                    nc.vector.reciprocal(rsb[:], sdb[:])
                    xn2 = work.tile([P, HID], F32, tag="xn2")
                    nc.vector.tensor_scalar(out=xn2[:], in0=o1f,
                                            scalar1=mvb[:, 0:1], scalar2=rsb[:],
                                            op0=OP.subtract, op1=OP.mult)
                    nc.vector.tensor_mul(xn2[:], xn2[:], ng_bc[i][:])
                    nc.vector.tensor_add(xn2[:], xn2[:], nb_bc[i][:])
                    # elu(x) = relu(x) + exp(min(x,0)) - 1
                    tneg = work.tile([P, HID], F32, tag="tneg")
                    nc.vector.tensor_scalar(out=tneg[:], in0=xn2[:],
                                            scalar1=0.0, scalar2=None,
                                            op0=OP.min)
                    e1 = work.tile([P, HID], F32, tag="e1")
                    nc.scalar.activation(e1[:], tneg[:], AF.Exp)
                    r1 = work.tile([P, HID], F32, tag="r1")
                    nc.scalar.activation(r1[:], xn2[:], AF.Relu)
                    xout = work.tile([P, HID], F32, tag="xout")
                    nc.vector.tensor_add(xout[:], r1[:], e1[:])
                    nc.vector.tensor_scalar(out=xout[:], in0=xout[:],
                                            scalar1=-1.0, scalar2=None,
                                            op0=OP.add)
                    nc.vector.tensor_add(xout[:], xout[:], xnm_sb[:, b, :])
                    if last:
                        nc.sync.dma_start(out=out[b * P:(b + 1) * P, :],
                                          in_=xout[:])
                    else:
                        nc.scalar.copy(xnm_sb[:, b, :], xout[:])
                        ptx2 = ps_tr.tile([P, P], F32, tag="m")
                        nc.tensor.transpose(ptx2[:], xout[:], identity[:])
                        nc.scalar.copy(xT_sb[:, b * P:(b + 1) * P], ptx2[:])
                        if KDEBUG and i == 0:
                            nc.sync.dma_start(out=dbg_x1[:, b, :], in_=xout[:])

    _split_waits(nc)
    return nc


# ---------------- public entry point ----------------
def kernel(**inputs):
    _install_ntff_hook()
    per_core, consts, meta = _prep(inputs)
    nc = _build(consts, meta)
    in_maps = [per_core[k] for k in range(NCORES)]
    res = run_bass_kernel_spmd(nc, in_maps, list(range(NCORES)),
                               trace=bool(int(__import__("os").environ.get(
                                   "KERNEL_TRACE", "0"))))
    kernel.last_exec_time_ns = res.exec_time_ns
    kernel.last_results = res
    outs = [res.results[k]["out"][:NL] for k in range(NCORES)]
    return np.concatenate(outs, axis=0).astype(np.float32)


kernel.last_exec_time_ns = None


# revision 18
# speedup vs baseline: 1.5369x; 1.1878x over previous
"""PlantCCCEncoder (3-layer GAT over 50k nodes / 800k edges) on 8 trn2 cores.

Strategy: edges sharded by dst node range (each core owns 6250 dst nodes and
all edges pointing into them -> segment softmax/sum are core-local, no
all-reduce). Node features replicated via a per-layer AllGather of the
src-projection table XS. Per-edge xs rows fetched with indirect DMA gathers;
xd and edge-feature terms come from PE matmuls (one-hot expansion) off
SBUF-resident tables, overlapped under the gather.

Self-contained: hardcodes all shapes; builds + runs the Bass program via
run_bass_kernel_spmd on cores 0-7.
"""
import sys
import types

sys.path.insert(0, "/opt/trn_rl_repo")

import numpy as np

import concourse.bass as bass
import concourse.tile as tile
from concourse import mybir
from concourse.bass_utils import run_bass_kernel_spmd

# ---------------- model dims (from the reference) ----------------
N = 50000
E = 800000
IN = 256
HID = 128
H = 4
C = 32
L = 3
REL = 100
RD = 16
EH = 32
LN_EPS = 1e-5

NCORES = 8
P = 128
NL = N // NCORES            # 6250 local dst nodes per core
NB = (NL + P - 1) // P      # 49 dst blocks
NLP = NB * P                # 6272 padded local nodes
NG = NCORES * NLP           # 50176 rows in the all-gathered XS table

F32 = mybir.dt.float32
F16 = mybir.dt.float16
I32 = mybir.dt.int32

PAD_COL = 200.0             # one-hot column id for padding edges (never matches)


# ---------------- NTFF profile hook (for trace timing) ----------------
def _install_ntff_hook():
    try:
        import antenv  # noqa: F401
        if "antenv.axon_hooks" in sys.modules:
            return
        mod = types.ModuleType("antenv.axon_hooks")
        _hook = [None]
        mod.set_axon_ntff_profile_hook = lambda h: _hook.__setitem__(0, h)
        mod.get_axon_ntff_profile_hook = lambda: _hook[0]
        sys.modules["antenv.axon_hooks"] = mod
        setattr(sys.modules["antenv"], "axon_hooks", mod)
        from trn_agent_boot.trn_boot import _ntff_profile_via_ctypes
        mod.set_axon_ntff_profile_hook(
            _ntff_profile_via_ctypes("/opt/axon/libaxon_pjrt.so"))
    except Exception:
        pass


# ---------------- walrus 1-wait-per-instruction workaround ----------------
def _split_waits(nc):
    def _get_si(ins):
        si = ins.sync_info
        return si() if callable(si) else si

    for f in nc.m.functions:
        for bb in f.blocks:
            insts = list(bb.instructions)
            out = []
            changed = False
            for ins in insts:
                if not isinstance(ins, mybir.InstEventSemaphore):
                    si = _get_si(ins)
                    if si is not None and len(si.on_wait) > 1:
                        waits = list(si.on_wait)
                        for k, w in enumerate(waits[1:]):
                            ev = mybir.InstEventSemaphore(
                                name=f"{ins.name}-ws{k}", engine=ins.engine,
                                ins=[], outs=[],
                                sync_info=mybir.SyncInfo(on_wait=[w], on_update=[]))
                            ev.bass_nofuse = True
                            out.append(ev)
                        ins.sync_info = mybir.SyncInfo(
                            on_wait=waits[:1], on_update=list(si.on_update))
                        changed = True
                out.append(ins)
            if changed:
                bb.instructions = out


# ---------------- host-side preprocessing ----------------
def _prep(inputs):
    x = np.asarray(inputs["x"], np.float32)
    edge_index = np.asarray(inputs["edge_index"]).astype(np.int64)
    edge_attr = np.asarray(inputs["edge_attr"], np.float32)

    g = {k: np.asarray(v, np.float32) for k, v in inputs.items()
         if k not in ("x", "edge_index", "edge_attr")}

    src = edge_index[0]
    dst = edge_index[1]

    # ---- parameter folding (host, parameters only) ----
    fus_W1 = g["fus_W1"]                     # [48, 32]
    A, B, Cc = fus_W1[:RD], fus_W1[RD:2 * RD], fus_W1[2 * RD:]
    dW2A = g["dist_W2"] @ A                  # [32, 32]
    cW2B = g["coexp_W2"] @ B                 # [32, 32]
    relp = g["rel_emb"] @ Cc                 # [100, 32]
    b_comb = (g["fus_b1"] + g["dist_b2"] @ A + g["coexp_b2"] @ B)  # [32]
    Wfold = np.stack([g["fus_W2"] @ g["Wedge"][i] for i in range(L)])      # [3,32,128]
    bias_ef = np.stack([g["fus_b2"] @ g["Wedge"][i] for i in range(L)])    # [3,128]

    # ---- edge sharding: sort by dst, shard by dst range, block by 128 ----
    order = np.argsort(dst, kind="stable")
    src_s, dst_s = src[order], dst[order]
    ea_s = edge_attr[order]

    core_of = dst_s // NL
    dl = dst_s % NL
    blk = dl // P
    col = dl % P

    # per (core, block) edge lists
    counts = np.zeros((NCORES, NB), np.int64)
    for k in range(NCORES):
        m = core_of == k
        bb = blk[m]
        cnt = np.bincount(bb, minlength=NB)
        counts[k] = cnt
    T_b = np.maximum(1, ((counts.max(axis=0) + P - 1) // P)).astype(np.int64)  # [NB]
    TT = int(T_b.sum())
    EP = TT * P
    tile_base = np.concatenate([[0], np.cumsum(T_b)])[:-1]  # first tile idx per block

    # slot assignment per core
    rng_rows = np.arange(N, dtype=np.int64)
    pad_row = (rng_rows // NL) * NLP + (rng_rows % NL)      # global node -> XS row

    per_core = []
    for k in range(NCORES):
        m = core_of == k
        e_src = src_s[m]
        e_col = col[m]
        e_blk = blk[m]
        e_d = ea_s[m, 0]
        e_c = ea_s[m, 1]
        e_rel = np.clip(ea_s[m, 2].astype(np.int64), 0, REL - 1)

        srcg = np.zeros(EP, np.int32)
        dstf = np.full(EP, PAD_COL, np.float32)
        logd = np.zeros(EP, np.float32)
        dT = np.zeros(EP, np.float32)
        cT = np.zeros(EP, np.float32)
        relT = np.zeros((32, EP), np.float32)

        # stable order within block
        ob = np.argsort(e_blk, kind="stable")
        e_src, e_col, e_blk = e_src[ob], e_col[ob], e_blk[ob]
        e_d, e_c, e_rel = e_d[ob], e_c[ob], e_rel[ob]
        # slot = (tile_base[b]*128 + position within block's slots)
        start = 0
        for b in range(NB):
            nb_ = counts[k, b]
            sl0 = tile_base[b] * P
            sl = np.arange(nb_) + sl0
            seg = slice(start, start + nb_)
            srcg[sl] = pad_row[e_src[seg]]
            dstf[sl] = e_col[seg]
            logd[sl] = np.log(e_d[seg])
            dT[sl] = e_d[seg]
            cT[sl] = e_c[seg]
            relT[:, sl] = relp[e_rel[seg]].T
            start += nb_

        # [128, TT] layouts: slot = ti*128 + p  ->  [p, ti]
        srcg2 = srcg.reshape(TT, P).T.copy()
        dstf2 = dstf.reshape(TT, P).T.copy()
        logd2 = logd.reshape(TT, P).T.copy()

        xk = np.zeros((NLP, IN), np.float32)
        xk[:NL] = x[k * NL:(k + 1) * NL]

        per_core.append(dict(
            x_in=xk, srcg=srcg2, dstf=dstf2, logd=logd2,
            dT=dT[None, :].astype(np.float16), cT=cT[None, :].astype(np.float16),
            relT=relT,
        ))

    def rep(row):
        return np.tile(np.asarray(row, np.float32)[None, :], (P, 1))

    consts = dict(
        identity=np.eye(P, dtype=np.float32),
        identity16=np.eye(P, dtype=np.float16),
        iota_row=np.tile(np.arange(P, dtype=np.float32), (P, 1)),
        ones32=np.ones((32, 1), np.float32),
        ones1x32=np.ones((1, 32), np.float32),
        in_g=rep(g["in_g"]), in_b=rep(g["in_b"]),
        proj_W=g["proj_W"], proj_b=rep(g["proj_b"]),
        dW1=g["dist_W1"].astype(np.float16), cW1=g["coexp_W1"].astype(np.float16),
        b1d=g["dist_b1"][:, None], b1c=g["coexp_b1"][:, None],
        dW2A=dW2A.astype(np.float16), cW2B=cW2B.astype(np.float16), b_comb=b_comb[:, None],
        fusg=g["fus_ln_g"][:, None], fusb=g["fus_ln_b"][:, None],
        Wsrc=g["Wsrc"], Wdst=g["Wdst"], Wfold=Wfold.astype(np.float16),
        bias_ef=np.stack([rep(bias_ef[i]) for i in range(L)]),
        att=np.stack([rep(g["att"][i].reshape(H * C)) for i in range(L)]),
        sscale=np.stack([rep(g["sscale"][i]) for i in range(L)]),
        ng=np.stack([rep(g["ng"][i]) for i in range(L)]),
        nb=np.stack([rep(g["nb"][i]) for i in range(L)]),
    )
    meta = dict(T_b=T_b.tolist(), tile_base=tile_base.tolist(), TT=TT, EP=EP)
    return per_core, consts, meta


# ---------------- device program ----------------
def _build(consts, meta):
    T_b, tile_base, TT, EP = (meta["T_b"], meta["tile_base"],
                              meta["TT"], meta["EP"])
    nc = bass.Bass(num_devices=NCORES)

    # per-core inputs
    x_in = nc.declare_dram_parameter("x_in", [NLP, IN], F32, isOutput=False)
    srcg = nc.declare_dram_parameter("srcg", [P, TT], I32, isOutput=False)
    dstf = nc.declare_dram_parameter("dstf", [P, TT], F32, isOutput=False)
    logd = nc.declare_dram_parameter("logd", [P, TT], F32, isOutput=False)
    dT = nc.declare_dram_parameter("dT", [1, EP], F16, isOutput=False)
    cT = nc.declare_dram_parameter("cT", [1, EP], F16, isOutput=False)
    relT = nc.declare_dram_parameter("relT", [32, EP], F32, isOutput=False)
    out = nc.declare_dram_parameter("out", [NLP, HID], F32, isOutput=True)
    import os as _os
    KDEBUG = bool(int(_os.environ.get("KDEBUG", "0")))
    if KDEBUG:
        dbg_h = nc.declare_dram_parameter("dbg_h", [32, EP], F32, isOutput=True)
        dbg_x0 = nc.declare_dram_parameter("dbg_x0", [P, NB, HID], F32, isOutput=True)
        dbg_xs = nc.declare_dram_parameter("dbg_xs", [NG, HID], F32, isOutput=True)
        dbg_xd = nc.declare_dram_parameter("dbg_xd", [P, NB, HID], F32, isOutput=True)
        dbg_x1 = nc.declare_dram_parameter("dbg_x1", [P, NB, HID], F32, isOutput=True)

    ct = {k: nc.inline_tensor(np.ascontiguousarray(v), name=f"c_{k}")
          for k, v in consts.items()}

    with tile.TileContext(nc) as tc:
        import contextlib
        ctx = contextlib.ExitStack()
        with ctx:
            sing = ctx.enter_context(tc.tile_pool(name="sing", bufs=1))
            work = ctx.enter_context(tc.tile_pool(name="work", bufs=4))
            wsm = ctx.enter_context(tc.tile_pool(name="wsm", bufs=4))
            ps_acc = ctx.enter_context(tc.tile_pool(name="psacc", bufs=2, space="PSUM"))
            ps_den = ctx.enter_context(tc.tile_pool(name="psden", bufs=2, space="PSUM"))
            ps_ef = ctx.enter_context(tc.tile_pool(name="psef", bufs=2, space="PSUM"))
            ps_m = ctx.enter_context(tc.tile_pool(name="psm", bufs=2, space="PSUM"))
            ps_tr = ps_m
            dram = ctx.enter_context(tc.tile_pool(name="dram", bufs=1, space="DRAM"))
            encp = ctx.enter_context(tc.tile_pool(name="encp", bufs=3))

            _uid = [0]

            def bc_load(mat_t, parts, width, tag=None):
                """Load a pre-replicated [parts, width] constant."""
                _uid[0] += 1
                t = sing.tile([parts, width], F32, tag=tag or f"bc{_uid[0]}")
                nc.sync.dma_start(out=t[:], in_=mat_t[:, :])
                return t

            # ---- persistent SBUF tiles ----
            identity = sing.tile([P, P], F32)
            nc.sync.dma_start(out=identity[:], in_=ct["identity"][:, :])
            identity16 = sing.tile([P, P], F16, tag="id16")
            nc.sync.dma_start(out=identity16[:], in_=ct["identity16"][:, :])
            iota_row = sing.tile([P, P], F32)
            nc.sync.dma_start(out=iota_row[:], in_=ct["iota_row"][:, :])
            srcg_sb = sing.tile([P, TT], I32)
            nc.sync.dma_start(out=srcg_sb[:], in_=srcg[:, :])
            dstf_sb = sing.tile([P, TT], F32)
            nc.sync.dma_start(out=dstf_sb[:], in_=dstf[:, :])
            logd_sb = sing.tile([P, TT], F32)
            nc.sync.dma_start(out=logd_sb[:], in_=logd[:, :])

            xT_sb = sing.tile([P, NLP], F32)          # x feat-major (lhsT source)
            xnm_sb = sing.tile([P, NB, HID], F32)     # x node-major  [:, b, :]
            XD_sb = sing.tile([P, NB, HID], F16)      # XD' node-major

            in_g_bc = bc_load(ct["in_g"], P, IN)
            in_b_bc = bc_load(ct["in_b"], P, IN)
            proj_b_bc = bc_load(ct["proj_b"], P, HID)
            projW_lo = sing.tile([P, HID], F32)
            nc.sync.dma_start(out=projW_lo[:], in_=ct["proj_W"][0:P, :])
            projW_hi = sing.tile([P, HID], F32)
            nc.sync.dma_start(out=projW_hi[:], in_=ct["proj_W"][P:IN, :])

            Wsrc_sb, Wdst_sb, Wfold_sb = [], [], []
            att_bc, ss_bc, ng_bc, nb_bc, bef_bc = [], [], [], [], []
            for i in range(L):
                t = sing.tile([HID, HID], F32, tag=f"wsrc{i}")
                nc.sync.dma_start(out=t[:], in_=ct["Wsrc"][i, :, :])
                Wsrc_sb.append(t)
                t = sing.tile([HID, HID], F32, tag=f"wdst{i}")
                nc.sync.dma_start(out=t[:], in_=ct["Wdst"][i, :, :])
                Wdst_sb.append(t)
                t = sing.tile([32, HID], F16, tag=f"wfold{i}")
                nc.sync.dma_start(out=t[:], in_=ct["Wfold"][i, :, :])
                Wfold_sb.append(t)
                att_bc.append(bc_load(ct["att"][i], P, H * C))
                ss_bc.append(bc_load(ct["sscale"][i], P, H))
                ng_bc.append(bc_load(ct["ng"][i], P, HID))
                nb_bc.append(bc_load(ct["nb"][i], P, HID))
                bef_bc.append(bc_load(ct["bias_ef"][i], P, HID))

            # encoder consts (feat-major; per-partition columns)
            dW1_sb = sing.tile([1, 32], F16)
            nc.sync.dma_start(out=dW1_sb[:], in_=ct["dW1"][:, :])
            cW1_sb = sing.tile([1, 32], F16)
            nc.sync.dma_start(out=cW1_sb[:], in_=ct["cW1"][:, :])
            b1d_sb = sing.tile([32, 1], F32)
            nc.sync.dma_start(out=b1d_sb[:], in_=ct["b1d"][:, :])
            b1c_sb = sing.tile([32, 1], F32)
            nc.sync.dma_start(out=b1c_sb[:], in_=ct["b1c"][:, :])
            dW2A_sb = sing.tile([32, 32], F16)
            nc.sync.dma_start(out=dW2A_sb[:], in_=ct["dW2A"][:, :])
            cW2B_sb = sing.tile([32, 32], F16)
            nc.sync.dma_start(out=cW2B_sb[:], in_=ct["cW2B"][:, :])
            bcomb_sb = sing.tile([32, 1], F32)
            nc.sync.dma_start(out=bcomb_sb[:], in_=ct["b_comb"][:, :])
            fusg_sb = sing.tile([32, 1], F32)
            nc.sync.dma_start(out=fusg_sb[:], in_=ct["fusg"][:, :])
            fusb_sb = sing.tile([32, 1], F32)
            nc.sync.dma_start(out=fusb_sb[:], in_=ct["fusb"][:, :])
            ones32_sb = sing.tile([32, 1], F32)
            nc.sync.dma_start(out=ones32_sb[:], in_=ct["ones32"][:, :])
            ones1x32_sb = sing.tile([1, 32], F32)
            nc.sync.dma_start(out=ones1x32_sb[:], in_=ct["ones1x32"][:, :])

            eps_ln = sing.tile([P, 1], F32)
            nc.vector.memset(eps_ln[:], LN_EPS)
            eps_f1 = sing.tile([1, 1], F32)
            nc.vector.memset(eps_f1[:], LN_EPS)

            # DRAM scratch
            hT_d = dram.tile([32, EP], F16)
            XSl_d = dram.tile([NLP, HID], F32)
            XSf_d = dram.tile([NG, HID], F32)

            AF = mybir.ActivationFunctionType
            OP = mybir.AluOpType

            import os as _os
            PHASES = int(_os.environ.get("KPHASES", "3"))

            # ================= Phase 1: edge encoder (feat-major) =================
            FE = 256
            nchunk = (EP + FE - 1) // FE if PHASES >= 1 else 0
            for ci in range(nchunk):
                w = min(FE, EP - ci * FE)
                s0 = ci * FE
                # h1 = relu(d (x) dW1 + b1)
                p1 = ps_ef.tile([32, FE], F32, tag="ef")
                dTt = encp.tile([1, FE], F16, tag="dTt")
                nc.sync.dma_start(out=dTt[:, :w], in_=dT[:, s0:s0 + w])
                cTt = encp.tile([1, FE], F16, tag="cTt")
                nc.sync.dma_start(out=cTt[:, :w], in_=cT[:, s0:s0 + w])
                relt = encp.tile([32, FE], F32, tag="relt")
                nc.sync.dma_start(out=relt[:, :w], in_=relT[:, s0:s0 + w])

                nc.tensor.matmul(p1[:, :w], lhsT=dW1_sb[:], rhs=dTt[:, :w],
                                 start=True, stop=True)
                h1d = encp.tile([32, FE], F16, tag="h1d")
                nc.scalar.activation(h1d[:, :w], p1[:, :w], AF.Relu,
                                     bias=b1d_sb[:], scale=1.0)
                p1b = ps_ef.tile([32, FE], F32, tag="ef")
                nc.tensor.matmul(p1b[:, :w], lhsT=cW1_sb[:], rhs=cTt[:, :w],
                                 start=True, stop=True)
                h1c = encp.tile([32, FE], F16, tag="h1c")
                nc.scalar.activation(h1c[:, :w], p1b[:, :w], AF.Relu,
                                     bias=b1c_sb[:], scale=1.0)
                # hpre = dW2A^T h1d + cW2B^T h1c  (+ rel + b_comb)
                p2 = ps_ef.tile([32, FE], F32, tag="ef")
                nc.tensor.matmul(p2[:, :w], lhsT=dW2A_sb[:], rhs=h1d[:, :w],
                                 start=True, stop=False)
                nc.tensor.matmul(p2[:, :w], lhsT=cW2B_sb[:], rhs=h1c[:, :w],
                                 start=False, stop=True)
                hpre = encp.tile([32, FE], F32, tag="hpre")
                nc.vector.tensor_add(hpre[:, :w], p2[:, :w], relt[:, :w])
                nc.scalar.activation(hpre[:, :w], hpre[:, :w], AF.Identity,
                                     bias=bcomb_sb[:], scale=1.0)
                # LN over the 32 partitions (PE column-sum trick)
                sq = encp.tile([32, FE], F32, tag="sq")
                nc.vector.tensor_mul(sq[:, :w], hpre[:, :w], hpre[:, :w])
                pm = ps_m.tile([64, FE], F32, tag="m")
                nc.tensor.matmul(pm[0:1, :w], lhsT=ones32_sb[:], rhs=hpre[:, :w],
                                 start=True, stop=True)
                nc.tensor.matmul(pm[32:33, :w], lhsT=ones32_sb[:], rhs=sq[:, :w],
                                 start=True, stop=True)
                stats = encp.tile([1, 2, FE], F32, tag="stats")
                nc.scalar.activation(stats[:, 0, :w], pm[0:1, :w], AF.Copy,
                                     scale=1.0 / 32.0)          # mean
                nc.scalar.activation(stats[:, 1, :w], pm[32:33, :w], AF.Copy,
                                     scale=1.0 / 32.0)          # E[x^2]
                var = encp.tile([1, FE], F32, tag="var")
                nc.vector.tensor_mul(var[:, :w], stats[:, 0, :w], stats[:, 0, :w])
                nc.vector.tensor_tensor(out=var[:, :w], in0=stats[:, 1, :w],
                                        in1=var[:, :w], op=OP.subtract)
                sd = encp.tile([1, FE], F32, tag="sd")
                nc.scalar.activation(sd[:, :w], var[:, :w], AF.Sqrt,
                                     bias=eps_f1[0:1, :], scale=1.0)
                rs = encp.tile([1, FE], F32, tag="rs")
                nc.vector.reciprocal(rs[:, :w], sd[:, :w])
                # broadcast mean/rs to 32 partitions via PE outer product
                pmb = ps_m.tile([64, FE], F32, tag="m")
                nc.tensor.matmul(pmb[0:32, :w], lhsT=ones1x32_sb[:],
                                 rhs=stats[:, 0, :w], start=True, stop=True)
                nc.tensor.matmul(pmb[32:64, :w], lhsT=ones1x32_sb[:],
                                 rhs=rs[:, :w], start=True, stop=True)
                hn = encp.tile([32, FE], F32, tag="hn")
                nc.vector.tensor_tensor(out=hn[:, :w], in0=hpre[:, :w],
                                        in1=pmb[0:32, :w], op=OP.subtract)
                nc.vector.tensor_mul(hn[:, :w], hn[:, :w], pmb[32:64, :w])
                nc.vector.tensor_scalar(out=hn[:, :w], in0=hn[:, :w],
                                        scalar1=fusg_sb[:], scalar2=fusb_sb[:],
                                        op0=OP.mult, op1=OP.add)
                hfin = encp.tile([32, FE], F16, tag="hfin")
                nc.scalar.activation(hfin[:, :w], hn[:, :w], AF.Relu)
                nc.sync.dma_start(out=hT_d[:, s0:s0 + w], in_=hfin[:, :w])

            # ================= Phase 2: input LN + projection =================
            for b in range(NB if PHASES >= 2 else 0):
                xt = work.tile([P, IN], F32, tag="xt")
                nc.sync.dma_start(out=xt[:], in_=x_in[b * P:(b + 1) * P, :])
                st6 = wsm.tile([P, 6], F32, tag="st6")
                nc.vector.bn_stats(out=st6[:], in_=xt[:])
                mv = wsm.tile([P, 2], F32, tag="mv")
                nc.vector.bn_aggr(out=mv[:], in_=st6[:])
                sd2 = wsm.tile([P, 1], F32, tag="sd2")
                nc.scalar.activation(sd2[:], mv[:, 1:2], AF.Sqrt,
                                     bias=eps_ln[:], scale=1.0)
                rs2 = wsm.tile([P, 1], F32, tag="rs2")
                nc.vector.reciprocal(rs2[:], sd2[:])
                xn = work.tile([P, IN], F32, tag="xn")
                nc.vector.tensor_scalar(out=xn[:], in0=xt[:],
                                        scalar1=mv[:, 0:1], scalar2=rs2[:],
                                        op0=OP.subtract, op1=OP.mult)
                nc.vector.tensor_mul(xn[:], xn[:], in_g_bc[:])
                nc.vector.tensor_add(xn[:], xn[:], in_b_bc[:])
                # transpose halves -> lhsT
                ptA = ps_tr.tile([P, P], F32, tag="m")
                nc.tensor.transpose(ptA[:], xn[:, 0:P], identity[:])
                tA = work.tile([P, P], F32, tag="tA")
                nc.scalar.copy(tA[:], ptA[:])
                ptB = ps_tr.tile([P, P], F32, tag="m")
                nc.tensor.transpose(ptB[:], xn[:, P:IN], identity[:])
                tB = work.tile([P, P], F32, tag="tB")
                nc.scalar.copy(tB[:], ptB[:])
                px = ps_m.tile([P, HID], F32, tag="m")
                nc.tensor.matmul(px[:], lhsT=tA[:], rhs=projW_lo[:],
                                 start=True, stop=False)
                nc.tensor.matmul(px[:], lhsT=tB[:], rhs=projW_hi[:],
                                 start=False, stop=True)
                x0 = work.tile([P, HID], F32, tag="x0")
                nc.vector.tensor_add(x0[:], px[:], proj_b_bc[:])
                nc.scalar.copy(xnm_sb[:, b, :], x0[:])
                ptx = ps_tr.tile([P, P], F32, tag="m")
                nc.tensor.transpose(ptx[:], x0[:], identity[:])
                nc.scalar.copy(xT_sb[:, b * P:(b + 1) * P], ptx[:])

            if KDEBUG:
                nc.gpsimd.dma_start(out=dbg_h[:, :], in_=hT_d[:, :])
                nc.sync.dma_start(out=dbg_x0[:, :, :], in_=xnm_sb[:, :, :])

            # ================= Phase 3: GAT layers =================
            for i in range(L if PHASES >= 3 else 0):
                last = i == L - 1
                # --- XS / XD' production ---
                for b in range(NB):
                    pxs = ps_m.tile([P, HID], F32, tag="m")
                    nc.tensor.matmul(pxs[:], lhsT=xT_sb[:, b * P:(b + 1) * P],
                                     rhs=Wsrc_sb[i][:], start=True, stop=True)
                    xs_st = work.tile([P, HID], F32, tag="xs_st")
                    nc.scalar.copy(xs_st[:], pxs[:])
                    nc.sync.dma_start(out=XSl_d[b * P:(b + 1) * P, :],
                                      in_=xs_st[:])
                    pxd = ps_ef.tile([P, HID], F32, tag="ef")
                    nc.tensor.matmul(pxd[:], lhsT=xT_sb[:, b * P:(b + 1) * P],
                                     rhs=Wdst_sb[i][:], start=True, stop=True)
                    nc.vector.tensor_add(XD_sb[:, b, :], pxd[:], bef_bc[i][:])

                nc.gpsimd.collective_compute(
                    "AllGather", OP.bypass,
                    replica_groups=[list(range(NCORES))],
                    ins=[XSl_d[:, :].opt()], outs=[XSf_d[:, :].opt()])
                if KDEBUG and i == 0:
                    nc.sync.dma_start(out=dbg_xs[:, :], in_=XSf_d[:, :])
                    nc.sync.dma_start(out=dbg_xd[:, :, :], in_=XD_sb[:, :, :])

                # --- edge pass ---
                for b in range(NB):
                    acc = ps_acc.tile([P, HID], F32, tag="acc")
                    accd = ps_den.tile([P, H], F32, tag="accd")
                    hTb = work.tile([32, 18 * P], F16, tag="hTb")
                    nc.sync.dma_start(
                        out=hTb[:, :T_b[b] * P],
                        in_=hT_d[:, tile_base[b] * P:(tile_base[b] + T_b[b]) * P])
                    for t in range(T_b[b]):
                        ti = tile_base[b] + t
                        first, lastt = t == 0, t == T_b[b] - 1
                        # gather xs rows
                        xs_t = work.tile([P, HID], F32, tag="xs_t")
                        nc.gpsimd.indirect_dma_start(
                            out=xs_t[:], out_offset=None,
                            in_=XSf_d[:, :],
                            in_offset=bass.IndirectOffsetOnAxis(
                                ap=srcg_sb[:, ti:ti + 1], axis=0))
                        # one-hot and transposed one-hot
                        oh = work.tile([P, P], F16, tag="oh")
                        nc.vector.tensor_tensor(
                            out=oh[:],
                            in0=dstf_sb[:, ti:ti + 1].to_broadcast([P, P]),
                            in1=iota_row[:], op=OP.is_equal)
                        ptr = ps_tr.tile([P, 1024], F16, tag="m")
                        nc.tensor.transpose(ptr[:, 0:P], oh[:], identity16[:])
                        ohT = work.tile([P, P], F16, tag="ohT")
                        nc.scalar.copy(ohT[:], ptr[:, 0:P])
                        # ef + xd -> psum
                        pef = ps_ef.tile([P, HID], F32, tag="ef")
                        nc.tensor.matmul(pef[:], lhsT=hTb[:, t * P:(t + 1) * P],
                                         rhs=Wfold_sb[i][:],
                                         start=True, stop=False)
                        nc.tensor.matmul(pef[:], lhsT=ohT[:], rhs=XD_sb[:, b, :],
                                         start=False, stop=True)
                        # pre-activation, tanh
                        pre = work.tile([P, HID], F32, tag="pre")
                        nc.vector.tensor_add(pre[:], xs_t[:], pef[:])
                        a_t = work.tile([P, HID], F32, tag="a_t")
                        nc.scalar.activation(a_t[:], pre[:], AF.Tanh)
                        # alpha = sum_c a*att
                        wa = work.tile([P, H, C], F32, tag="wa")
                        nc.vector.tensor_mul(
                            wa[:].rearrange("p h c -> p (h c)"), a_t[:],
                            att_bc[i][:])
                        alpha = wsm.tile([P, H], F32, tag="alpha")
                        nc.vector.reduce_sum(out=alpha[:], in_=wa[:, :, :],
                                             axis=mybir.AxisListType.X)
                        if not last:
                            dec = wsm.tile([P, H], F32, tag="dec")
                            nc.vector.tensor_scalar(
                                out=dec[:], in0=ss_bc[i][:],
                                scalar1=logd_sb[:, ti:ti + 1], scalar2=None,
                                op0=OP.mult)
                            nc.scalar.activation(dec[:], dec[:], AF.Exp)
                            nc.vector.tensor_mul(alpha[:], alpha[:], dec[:])
                        ea = wsm.tile([P, H], F16, tag="ea")
                        nc.scalar.activation(ea[:], alpha[:], AF.Exp)
                        # msg = xs * ea (per-head broadcast)
                        msg = work.tile([P, H, C], F16, tag="msg")
                        nc.vector.tensor_tensor(
                            out=msg[:, :, :],
                            in0=xs_t[:].rearrange("p (h c) -> p h c", h=H),
                            in1=ea[:].unsqueeze(2).to_broadcast([P, H, C]),
                            op=OP.mult)
                        # segment accumulate
                        nc.tensor.matmul(acc[:], lhsT=oh[:],
                                         rhs=msg[:].rearrange("p h c -> p (h c)"),
                                         start=first, stop=lastt)
                        nc.tensor.matmul(accd[:], lhsT=oh[:],
                                         rhs=ea[:], start=first, stop=lastt)

                    # --- block epilogue ---
                    den = wsm.tile([P, H], F32, tag="den")
                    nc.vector.tensor_scalar(out=den[:], in0=accd[:],
                                            scalar1=1e-8, scalar2=None,
                                            op0=OP.add)
                    rec = wsm.tile([P, H], F32, tag="rec")
                    nc.vector.reciprocal(rec[:], den[:])
                    o1 = work.tile([P, H, C], F32, tag="o1")
                    nc.vector.tensor_tensor(
                        out=o1[:, :, :],
                        in0=acc[:].rearrange("p (h c) -> p h c", h=H),
                        in1=rec[:].unsqueeze(2).to_broadcast([P, H, C]),
                        op=OP.mult)
                    o1f = o1[:].rearrange("p h c -> p (h c)")
                    st6b = wsm.tile([P, 6], F32, tag="st6b")
                    nc.vector.bn_stats(out=st6b[:], in_=o1f)
                    mvb = wsm.tile([P, 2], F32, tag="mvb")
                    nc.vector.bn_aggr(out=mvb[:], in_=st6b[:])
                    sdb = wsm.tile([P, 1], F32, tag="sdb")
                    nc.scalar.activation(sdb[:], mvb[:, 1:2], AF.Sqrt,
                                         bias=eps_ln[:], scale=1.0)
                    rsb = wsm.tile([P, 1], F32, tag="rsb")
                    nc.vector.reciprocal(rsb[:], sdb[:])
                    xn2 = work.tile([P, HID], F32, tag="xn2")
                    nc.vector.tensor_scalar(out=xn2[:], in0=o1f,
                                            scalar1=mvb[:, 0:1], scalar2=rsb[:],
                                            op0=OP.subtract, op1=OP.mult)
                    nc.vector.tensor_mul(xn2[:], xn2[:], ng_bc[i][:])
                    nc.vector.tensor_add(xn2[:], xn2[:], nb_bc[i][:])
                    # elu(x) = relu(x) + exp(min(x,0)) - 1
                    tneg = work.tile([P, HID], F32, tag="tneg")
                    nc.vector.tensor_scalar(out=tneg[:], in0=xn2[:],
                                            scalar1=0.0, scalar2=None,
                                            op0=OP.min)
                    e1 = work.tile([P, HID], F32, tag="e1")
                    nc.scalar.activation(e1[:], tneg[:], AF.Exp)
                    r1 = work.tile([P, HID], F32, tag="r1")
                    nc.scalar.activation(r1[:], xn2[:], AF.Relu)
                    xout = work.tile([P, HID], F32, tag="xout")
                    nc.vector.tensor_add(xout[:], r1[:], e1[:])
                    nc.vector.tensor_scalar(out=xout[:], in0=xout[:],
                                            scalar1=-1.0, scalar2=None,
                                            op0=OP.add)
                    nc.vector.tensor_add(xout[:], xout[:], xnm_sb[:, b, :])
                    if last:
                        nc.sync.dma_start(out=out[b * P:(b + 1) * P, :],
                                          in_=xout[:])
                    else:
                        nc.scalar.copy(xnm_sb[:, b, :], xout[:])
                        ptx2 = ps_tr.tile([P, P], F32, tag="m")
                        nc.tensor.transpose(ptx2[:], xout[:], identity[:])
                        nc.scalar.copy(xT_sb[:, b * P:(b + 1) * P], ptx2[:])
                        if KDEBUG and i == 0:
                            nc.sync.dma_start(out=dbg_x1[:, b, :], in_=xout[:])

    _split_waits(nc)
    return nc


# ---------------- public entry point ----------------
def kernel(**inputs):
    _install_ntff_hook()
    per_core, consts, meta = _prep(inputs)
    nc = _build(consts, meta)
    in_maps = [per_core[k] for k in range(NCORES)]
    res = run_bass_kernel_spmd(nc, in_maps, list(range(NCORES)),
                               trace=bool(int(__import__("os").environ.get(
                                   "KERNEL_TRACE", "0"))))
    kernel.last_exec_time_ns = res.exec_time_ns
    kernel.last_results = res
    outs = [res.results[k]["out"][:NL] for k in range(NCORES)]
    return np.concatenate(outs, axis=0).astype(np.float32)


kernel.last_exec_time_ns = None
